# revision 8
# baseline (speedup 1.0000x reference)
"""DPB (dynamic position bias) window attention kernel for Trainium2.

Contract: kernel(**inputs) takes the FULL unsharded inputs (numpy) and
returns the FULL output, running a Bass/Tile kernel over 8 NeuronCores
(pure data parallel over the window-batch dim).

Hardcoded problem shapes:
  x    (3136, 64, 256) f32   -> 392 windows / core
  mask (49, 64, 64) f32      (zeros in practice; general path supported)
  out  (3136, 64, 256) f32

Design (v3):
  - scores computed TRANSPOSED (S^T[k, q], keys on partitions) so the
    attention probabilities are already key-major for the AV matmul —
    no PE transposes of P needed.
  - softmax denominator z comes for free from the AV matmul: V is
    augmented with a ones column (33-wide head blocks), so out[q, 32]
    accumulates sum_k E[k, q].
  - normalization deferred to after AV: O~ and z both land q-major in
    PSUM; one cheap reciprocal over 32 strided columns + one fused
    multiply-evacuate produce the normalized context.
  - O is transposed back to feature-major (8 small PE transposes) for
    the output projection.
  - exp(rpb) is folded in post-exp with a j-replicated bf16 table
    (non-broadcast APs keep the DVE in 2x mode).
  - y is stored to HBM in bf16 (halves output DMA); host upcasts.
"""

import sys

sys.path.insert(0, "/opt/trn_rl_repo")

import numpy as np
import ml_dtypes

import concourse.bass as bass
import concourse.tile as tile
from concourse import bacc, mybir
from concourse.masks import make_identity

BF16 = mybir.dt.bfloat16
F32 = mybir.dt.float32
AF = mybir.ActivationFunctionType
ALU = mybir.AluOpType

# ---- problem constants ----------------------------------------------------
DIM = 256
HEADS = 8
HD = 32
NTOK = 64
NW49 = 49
BATCH = 64
B_ = BATCH * NW49          # 3136
NCORES = 8
WPC = B_ // NCORES         # 392 windows per core
TPC = WPC * NTOK           # 25088 tokens per core
CHW = 8                    # windows per chunk
NCHUNK = WPC // CHW        # 49
SCALE = HD ** -0.5


def _np_bf16(a):
    return np.asarray(a, dtype=ml_dtypes.bfloat16)


# ---- host-side DPB MLP + relative-position tables --------------------------
def _host_rpb(inputs):
    """rpb[h, q, k] = p3[rel_idx[q, k], h] where p3 = DPB MLP(biases)."""
    f = lambda k: np.asarray(inputs[k], np.float32)
    biases = f("biases")            # (225, 2)
    eps = 1e-5

    def ln(x, g, b):
        m = x.mean(-1, keepdims=True)
        v = ((x - m) ** 2).mean(-1, keepdims=True)
        return (x - m) / np.sqrt(v + eps) * g + b

    p = biases @ f("pos_proj_w").T + f("pos_proj_b")
    p = np.maximum(ln(p, f("ln1_g"), f("ln1_b")), 0.0) @ f("fc1_w").T + f("fc1_b")
    p = np.maximum(ln(p, f("ln2_g"), f("ln2_b")), 0.0) @ f("fc2_w").T + f("fc2_b")
    p = np.maximum(ln(p, f("ln3_g"), f("ln3_b")), 0.0) @ f("fc3_w").T + f("fc3_b")
    # p: (225, HEADS)
    rel_idx = np.asarray(inputs["rel_idx"], np.int64)      # (64, 64)
    rpb = p[rel_idx]                                        # (q, k, h)
    return np.transpose(rpb, (2, 0, 1))                     # (h, q, k)


def _host_tables(inputs, mask_nonzero):
    """Fast path: tfast [128, 2048] bf16 with
         tfast[k2, r*512 + j*128 + hh*64 + q] = exp(rpb[hh*4+r, q, k2%64])
       (k duplicated over the two 64-partition window halves, replicated
       over j so the multiply AP is non-broadcast -> DVE 2x mode).
       Mask path: etmask [64, 49*4*128] bf16 with
         etmask[k, (t*4 + r)*128 + hh*64 + q] = exp(rpb + mask[t])."""
    rpb = _host_rpb(inputs)                                 # (h, q, k)
    if not mask_nonzero:
        t = np.empty((64, 2048), np.float32)
        for r in range(4):
            for hh in range(2):
                h = hh * 4 + r
                e = np.exp(rpb[h].T)                        # (k, q)
                for j in range(4):
                    t[:, r * 512 + j * 128 + hh * 64:
                         r * 512 + j * 128 + hh * 64 + 64] = e
        return _np_bf16(np.concatenate([t, t], axis=0)), None
    mask = np.asarray(inputs["mask"], np.float32)           # (49, 64, 64)
    em = np.empty((64, NW49 * 4 * 128), np.float32)
    for t49 in range(NW49):
        for r in range(4):
            for hh in range(2):
                h = hh * 4 + r
                e = np.exp(rpb[h] + mask[t49]).T            # (k, q)
                base = (t49 * 4 + r) * 128 + hh * 64
                em[:, base:base + 64] = e
    return None, _np_bf16(em)


# ---- device kernel builder -------------------------------------------------
def _build(mask_nonzero, qkvb_nonzero, projb_nonzero, nchunk=NCHUNK):
    nc = bacc.Bacc("TRN2", target_bir_lowering=False, debug=False)

    x_d = nc.dram_tensor("x", (2, 128, TPC), BF16, kind="ExternalInput")
    y_d = nc.dram_tensor("y", (TPC, DIM), BF16, kind="ExternalOutput")
    wqk_d = nc.dram_tensor("wqk", (2, 128, 512), BF16, kind="ExternalInput")
    wv_d = nc.dram_tensor("wv", (2, 128, 256), BF16, kind="ExternalInput")
    pw_d = nc.dram_tensor("pw", (2, 128, 256), BF16, kind="ExternalInput")
    vtmpl_d = nc.dram_tensor("vtmpl", (128, 2 * 4 * 264), BF16, kind="ExternalInput")
    if mask_nonzero:
        e_d = nc.dram_tensor("etab", (64, NW49 * 4 * 128), BF16, kind="ExternalInput")
    else:
        e_d = nc.dram_tensor("etab", (128, 2048), BF16, kind="ExternalInput")
    if qkvb_nonzero:
        qkb_d = nc.dram_tensor("qkb", (4, 128), F32, kind="ExternalInput")
        vb_d = nc.dram_tensor("vb", (128, 264), BF16, kind="ExternalInput")
    if projb_nonzero:
        yb_d = nc.dram_tensor("yb", (128, 256), F32, kind="ExternalInput")

    with tile.TileContext(nc) as tc:
        with (
            tc.tile_pool(name="setup", bufs=1) as setup,
            tc.tile_pool(name="xts", bufs=3) as xts,
            tc.tile_pool(name="qks", bufs=3) as qks,
            tc.tile_pool(name="ets", bufs=2) as ets,
            tc.tile_pool(name="avqs", bufs=2) as avqs,
            tc.tile_pool(name="avs", bufs=2) as avs,
            tc.tile_pool(name="ys", bufs=3) as ysp,
            tc.tile_pool(name="dst", bufs=4) as dst,
            tc.tile_pool(name="pp_a", bufs=2, space="PSUM") as pp_a,
            tc.tile_pool(name="pp_sc", bufs=2, space="PSUM") as pp_sc,
            tc.tile_pool(name="pp_av", bufs=1, space="PSUM") as pp_av,
        ):
            # ---- one-time setup ----
            ident = setup.tile([128, 128], BF16)
            make_identity(nc, ident)

            wqk = setup.tile([128, 2, 512], BF16)
            nc.gpsimd.dma_start(
                out=wqk,
                in_=bass.AP(tensor=wqk_d, offset=0,
                            ap=[[512, 128], [128 * 512, 2], [1, 512]]),
            )
            wv = setup.tile([128, 2, 256], BF16)
            nc.gpsimd.dma_start(
                out=wv,
                in_=bass.AP(tensor=wv_d, offset=0,
                            ap=[[256, 128], [128 * 256, 2], [1, 256]]),
            )
            pw = setup.tile([128, 2, 256], BF16)
            nc.gpsimd.dma_start(
                out=pw,
                in_=bass.AP(tensor=pw_d, offset=0,
                            ap=[[256, 128], [128 * 256, 2], [1, 256]]),
            )
            # v_aug: persistent double-buffered (h, 33)-block v staging with
            # pre-seeded ones columns (from the host template)
            v_aug = setup.tile([128, 2, 4, 264], BF16)
            nc.gpsimd.dma_start(out=v_aug, in_=vtmpl_d.ap())
            if mask_nonzero:
                etab = setup.tile([64, NW49 * 4 * 128], BF16)
                nc.gpsimd.dma_start(out=etab, in_=e_d.ap())
            else:
                etab = setup.tile([128, 2048], BF16)
                nc.gpsimd.dma_start(out=etab, in_=e_d.ap())
            if qkvb_nonzero:
                qkb = setup.tile([128, 4], F32)
                nc.gpsimd.dma_start(
                    out=qkb,
                    in_=bass.AP(tensor=qkb_d, offset=0, ap=[[1, 128], [128, 4]]),
                )
                vb = setup.tile([128, 264], BF16)
                nc.gpsimd.dma_start(out=vb, in_=vb_d.ap())
            if projb_nonzero:
                yb = setup.tile([128, 256], F32)
                nc.gpsimd.dma_start(out=yb, in_=yb_d.ap())

            # ---- software-pipelined main loop (8 windows / 512 tok per chunk)
            # PE order per iteration: qkv(c) | AV(c-1) | scores(c) | O^T+proj(c-1)
            # so the PE never idles on the chunk-tail DVE chain (keeps HAM warm).

            def head1(c):
                tok0 = c * 512
                par = c % 2
                st = {}
                xt = xts.tile([128, 2, 512], BF16)
                nc.sync.dma_start(
                    out=xt,
                    in_=bass.AP(tensor=x_d, offset=tok0,
                                ap=[[TPC, 128], [128 * TPC, 2], [1, 512]]),
                )
                # qkT = Wqk.T @ xT : 4 m-tiles (q0 q1 k0 k1)
                qk = qks.tile([128, 4, 512], BF16)
                for m in (0, 2, 1, 3):
                    qkp = pp_a.tile([128, 512], F32, name="qkp", tag="mm")
                    for kk in range(2):
                        nc.tensor.matmul(
                            qkp,
                            wqk[:, kk, 128 * m:128 * (m + 1)],
                            xt[:, kk, :],
                            start=(kk == 0),
                            stop=(kk == 1),
                        )
                    if qkvb_nonzero:
                        if m < 2:
                            nc.scalar.activation(
                                qk[:, m, :], qkp, AF.Copy, bias=qkb[:, m:m + 1]
                            )
                        else:
                            nc.vector.tensor_scalar_add(
                                qk[:, m, :], qkp, qkb[:, m:m + 1]
                            )
                    else:
                        if m < 2:
                            nc.scalar.copy(qk[:, m, :], qkp)
                        else:
                            nc.vector.tensor_copy(qk[:, m, :], qkp)

                # v (token-major) -> v_aug (h, 33)-blocks, ones col persists
                for jp in range(2):
                    vp = pp_a.tile([128, 512], F32, name="vp", tag="mm")
                    for j2 in range(2):
                        j = jp * 2 + j2
                        for kk in range(2):
                            nc.tensor.matmul(
                                vp[:, 256 * j2:256 * (j2 + 1)],
                                xt[:, kk, 128 * j:128 * (j + 1)],
                                wv[:, kk, :],
                                start=(kk == 0),
                                stop=(kk == 1),
                            )
                    vdst = bass.AP(
                        tensor=v_aug.tensor,
                        offset=v_aug.offset + par * (4 * 264) + jp * 2 * 264,
                        ap=[v_aug.ap[0], [264, 2], [33, 8], [1, 32]],
                    )
                    vsrc = vp.rearrange("p (j2 h d) -> p j2 h d", j2=2, h=8)
                    if qkvb_nonzero:
                        vb_ap = bass.AP(
                            tensor=vb.tensor, offset=vb.offset,
                            ap=[vb.ap[0], [0, 2], [33, 8], [1, 32]],
                        )
                        nc.vector.tensor_tensor(vdst, vsrc, vb_ap, ALU.add)
                    else:
                        nc.vector.tensor_copy(vdst, vsrc)
                st["qk"] = qk
                return st

            def head2(c, st):
                qk = st["qk"]
                # scores^T: per-r psum [128 = 2win keys, 512 = (j, hh, q)];
                # r-serial so exp(r) frees its bank during r+1's matmuls
                et = ets.tile([128, 2048], BF16)
                for r in range(4):
                    sc = pp_sc.tile([128, 512], F32, name=f"sc{r}", tag="sc")
                    scv = sc.rearrange("p (j hh q) -> p j hh q", j=4, hh=2)
                    for j in range(4):
                        for hh in range(2):
                            for win in range(2):
                                base = 128 * j + 64 * win
                                nc.tensor.matmul(
                                    scv[64 * win:64 * (win + 1), j, hh, :],
                                    qk[32 * r:32 * r + 32, 2 + hh,
                                       base:base + 64],
                                    qk[32 * r:32 * r + 32, hh,
                                       base:base + 64],
                                    tile_position=(32 * r, 64 * win),
                                )
                    nc.scalar.activation(
                        et[:, 512 * r:512 * (r + 1)], sc, AF.Exp
                    )

                # fold exp(rpb [+ mask]) multiplicatively
                if not mask_nonzero:
                    for r in range(4):
                        sl = et[:, 512 * r:512 * (r + 1)]
                        nc.vector.tensor_tensor(
                            sl, sl, etab[:, 512 * r:512 * (r + 1)], ALU.mult
                        )
                else:
                    for w in range(CHW):
                        t49 = (c * CHW + w) % NW49
                        j, win = w // 2, w % 2
                        for r in range(4):
                            sl = et[64 * win:64 * (win + 1),
                                    512 * r + 128 * j:512 * r + 128 * j + 128]
                            e_ap = bass.AP(
                                tensor=etab.tensor,
                                offset=etab.offset + (t49 * 4 + r) * 128,
                                ap=[etab.ap[0], [1, 128]],
                            )
                            nc.vector.tensor_tensor(sl, sl, e_ap, ALU.mult)
                st["et"] = et

            def tail_a(c, st):
                et = st["et"]
                par = c % 2
                # AV (+z): stationary E^T slices, stream v_aug (33-wide);
                # win innermost alternates PE row groups
                av = pp_av.tile([128, 2048], F32, name="av", tag="av")
                for r in range(4):
                    for j in range(4):
                        for hh in range(2):
                            h8 = hh * 4 + r
                            ebase = 512 * r + 128 * j + 64 * hh
                            for win in range(2):
                                nc.tensor.matmul(
                                    av[64 * win:64 * (win + 1),
                                       j * 512 + 33 * h8:j * 512 + 33 * h8 + 33],
                                    et[64 * win:64 * (win + 1), ebase:ebase + 64],
                                    v_aug[64 * win:64 * (win + 1), par, j,
                                          33 * h8:33 * h8 + 33],
                                    tile_position=(64 * win, 64 * win),
                                )

                # reciprocal of z (strided cols 512j + 33h + 32)
                rz = dst.tile([128, 4, 8], F32)
                z_ap = bass.AP(
                    tensor=av.tensor, offset=av.offset + 32,
                    ap=[av.ap[0], [512, 4], [33, 8]],
                )
                nc.vector.reciprocal_approx_fast(rz, z_ap)

                # normalize + evacuate to compact q-major bf16
                avq = avqs.tile([128, 1024], BF16)
                avq_v = avq.rearrange("p (j h d) -> p j h d", j=4, h=8)
                o_ap = bass.AP(
                    tensor=av.tensor, offset=av.offset,
                    ap=[av.ap[0], [512, 4], [33, 8], [1, 32]],
                )
                rz_ap = bass.AP(
                    tensor=rz.tensor, offset=rz.offset,
                    ap=[rz.ap[0], [8, 4], [1, 8], [0, 32]],
                )
                nc.vector.tensor_tensor(avq_v, o_ap, rz_ap, ALU.mult)
                st["avq"] = avq

            def tail_b(c, st):
                avq = st["avq"]
                tok0 = c * 512
                # transpose O back to feature-major for proj
                av_sb = avs.tile([128, 2, 512], BF16)
                for tp in range(2):
                    tr = pp_a.tile([128, 512], BF16, name=f"tr{tp}", tag="mm")
                    for i in range(4):
                        j = tp * 2 + i // 2
                        kk = i % 2
                        nc.tensor.transpose(
                            tr[:, 128 * i:128 * (i + 1)],
                            avq[:, 256 * j + 128 * kk:256 * j + 128 * kk + 128],
                            ident,
                        )
                    # tr cols = (j2, kk, q) -> av_sb[., kk, 128j + q]
                    tdst = bass.AP(
                        tensor=av_sb.tensor,
                        offset=av_sb.offset + tp * 256,
                        ap=[av_sb.ap[0], [128, 2], [512, 2], [1, 128]],
                    )
                    tsrc = tr.rearrange("p (j2 kk q) -> p j2 kk q", j2=2, kk=2)
                    if tp == 0:
                        nc.vector.tensor_copy(tdst, tsrc)
                    else:
                        nc.scalar.copy(tdst, tsrc)

                # proj
                y_sb = ysp.tile([128, 4, 256], BF16)
                for yp2 in range(2):
                    yp = pp_a.tile([128, 512], F32, name=f"yp{yp2}", tag="mm")
                    for l in range(2):
                        jj = yp2 * 2 + l
                        for kk in range(2):
                            nc.tensor.matmul(
                                yp[:, 256 * l:256 * (l + 1)],
                                av_sb[:, kk, 128 * jj:128 * (jj + 1)],
                                pw[:, kk, :],
                                start=(kk == 0),
                                stop=(kk == 1),
                            )
                    if projb_nonzero:
                        yb_ap = bass.AP(
                            tensor=yb.tensor, offset=yb.offset,
                            ap=[yb.ap[0], [0, 2], [1, 256]],
                        )
                        ydst = y_sb[:, 2 * yp2:2 * yp2 + 2, :]
                        nc.vector.tensor_tensor(
                            ydst, yp.rearrange("p (l f) -> p l f", l=2),
                            yb_ap, ALU.add,
                        )
                    else:
                        nc.scalar.copy(y_sb[:, 2 * yp2:2 * yp2 + 2, :], yp)

                nc.sync.dma_start(
                    out=bass.AP(tensor=y_d, offset=tok0 * DIM,
                                ap=[[DIM, 128], [128 * DIM, 4], [1, DIM]]),
                    in_=y_sb,
                )

            states = {0: head1(0)}
            head2(0, states[0])
            tail_a(0, states[0])
            for c in range(1, nchunk):
                states[c] = head1(c)
                tail_b(c - 1, states[c - 1])
                head2(c, states[c])
                tail_a(c, states[c])
                del states[c - 1]
            tail_b(nchunk - 1, states[nchunk - 1])

    nc.compile()
    return nc


# ---- execution --------------------------------------------------------------
_CACHE = {}


def _get_runner(mask_nonzero, qkvb_nonzero, projb_nonzero, nchunk=NCHUNK):
    key = (mask_nonzero, qkvb_nonzero, projb_nonzero, nchunk)
    if key in _CACHE:
        return _CACHE[key]

    nc = _build(mask_nonzero, qkvb_nonzero, projb_nonzero, nchunk)

    import jax
    import jax.numpy as jnp
    from jax.sharding import Mesh, PartitionSpec
    from jax.experimental.shard_map import shard_map
    from concourse import bass2jax
    from concourse.bass2jax import _bass_exec_p, install_neuronx_cc_hook

    install_neuronx_cc_hook()

    partition_name = (
        nc.partition_id_tensor.name if nc.partition_id_tensor else None
    )
    in_names, out_names, out_avals, zero_outs = [], [], [], []
    for alloc in nc.m.functions[0].allocations:
        if not isinstance(alloc, mybir.MemoryLocationSet):
            continue
        name = alloc.memorylocations[0].name
        if alloc.kind == "ExternalInput":
            if name != partition_name:
                in_names.append(name)
        elif alloc.kind == "ExternalOutput":
            shape = tuple(alloc.tensor_shape)
            dtype = mybir.dt.np(alloc.dtype)
            out_names.append(name)
            out_avals.append(jax.core.ShapedArray(shape, dtype))
            zero_outs.append(np.zeros(shape, dtype))
    n_params = len(in_names)
    n_outs = len(out_avals)
    all_in_names = list(in_names) + list(out_names)
    if partition_name is not None:
        all_in_names.append(partition_name)

    def _body(*args):
        operands = list(args)
        if partition_name is not None:
            operands.append(bass2jax.partition_id_tensor())
        outs = _bass_exec_p.bind(
            *operands,
            out_avals=tuple(out_avals),
            in_names=tuple(all_in_names),
            out_names=tuple(out_names),
            lowering_input_output_aliases=(),
            sim_require_finite=True,
            sim_require_nnan=True,
            nc=nc,
        )
        return tuple(outs)

    devices = jax.devices()[:NCORES]
    mesh = Mesh(np.asarray(devices), ("core",))
    donate = tuple(range(n_params, n_params + n_outs))
    sharded = jax.jit(
        shard_map(
            _body, mesh=mesh,
            in_specs=(PartitionSpec("core"),) * (n_params + n_outs),
            out_specs=(PartitionSpec("core"),) * n_outs,
            check_rep=False,
        ),
        donate_argnums=donate,
        keep_unused=True,
    )

    from jax.sharding import NamedSharding

    shard = NamedSharding(mesh, PartitionSpec("core"))
    zero_shapes = [
        ((NCORES * z.shape[0], *z.shape[1:]), z.dtype) for z in zero_outs
    ]
    make_zeros = jax.jit(
        lambda: tuple(jnp.zeros(s, d) for s, d in zero_shapes),
        out_shardings=(shard,) * n_outs,
    )

    def _concat(in_maps):
        return [
            np.concatenate([np.asarray(in_maps[c][nm]) for c in range(NCORES)], axis=0)
            for nm in in_names
        ]

    def run(in_maps):
        out_arrs = sharded(*_concat(in_maps), *make_zeros())
        out = np.asarray(out_arrs[out_names.index("y")])
        return out.reshape(NCORES, TPC, DIM)

    def bench(in_maps, iters=8):
        import time as _time

        dev_in = [jax.device_put(a, shard) for a in _concat(in_maps)]
        jax.block_until_ready(dev_in)
        outs = sharded(*dev_in, *make_zeros())
        jax.block_until_ready(outs)
        ts = []
        for _ in range(iters):
            t0 = _time.perf_counter()
            outs = sharded(*dev_in, *make_zeros())
            jax.block_until_ready(outs)
            ts.append(_time.perf_counter() - t0)
        return min(ts), ts

    def bench_repeat(in_maps, reps, iters=6):
        """Async-chain `reps` dispatches (output ping-pongs into the donated
        slot) and block once; median over iters."""
        import time as _time

        dev_in = [jax.device_put(a, shard) for a in _concat(in_maps)]
        jax.block_until_ready(dev_in)
        outs = sharded(*dev_in, *make_zeros())
        jax.block_until_ready(outs)
        ts = []
        for _ in range(iters):
            outs = sharded(*dev_in, *make_zeros())
            jax.block_until_ready(outs)
            t0 = _time.perf_counter()
            for _ in range(reps):
                outs = sharded(*dev_in, *outs)
            jax.block_until_ready(outs)
            ts.append(_time.perf_counter() - t0)
        ts.sort()
        return ts[len(ts) // 2], ts

    run.bench = bench
    run.bench_repeat = bench_repeat
    _CACHE[key] = run
    return run


def _prep_in_maps(inputs):
    x = np.asarray(inputs["x"], np.float32).reshape(B_ * NTOK, DIM)
    qkv_w = np.asarray(inputs["qkv_w"], np.float32)
    qkv_b = np.asarray(inputs["qkv_b"], np.float32)
    proj_w = np.asarray(inputs["proj_w"], np.float32)
    proj_b = np.asarray(inputs["proj_b"], np.float32)
    mask_nonzero = bool(np.any(np.asarray(inputs["mask"]) != 0))
    qkvb_nonzero = bool(np.any(qkv_b != 0))
    projb_nonzero = bool(np.any(proj_b != 0))

    wqk_f = qkv_w[:512].copy()
    wqk_f[:256] *= SCALE                       # fold q scale into Wq
    wqk = _np_bf16(wqk_f.T.reshape(2, 128, 512))
    wv = _np_bf16(qkv_w[512:].T.reshape(2, 128, 256))
    pw = _np_bf16(proj_w.T.reshape(2, 128, 256))

    tfast, etmask = _host_tables(inputs, mask_nonzero)
    etab = etmask if mask_nonzero else tfast

    # v_aug template: zeros with ones in the 33rd column of each head block
    vtmpl = np.zeros((128, 2, 4, 8, 33), np.float32)
    vtmpl[:, :, :, :, 32] = 1.0
    vtmpl = _np_bf16(vtmpl.reshape(128, 2 * 4 * 264))

    shared = {"wqk": wqk, "wv": wv, "pw": pw, "etab": etab, "vtmpl": vtmpl}
    if qkvb_nonzero:
        qkb_f = qkv_b[:512].copy()
        qkb_f[:256] *= SCALE
        shared["qkb"] = qkb_f.reshape(4, 128).astype(np.float32)
        vb_aug = np.zeros((128, 8, 33), np.float32)
        vb_aug[:, :, :32] = qkv_b[512:].reshape(1, 8, 32)
        shared["vb"] = _np_bf16(vb_aug.reshape(128, 264))
    if projb_nonzero:
        shared["yb"] = np.broadcast_to(proj_b, (128, 256)).copy().astype(np.float32)

    in_maps = []
    for c in range(NCORES):
        m = dict(shared)
        xs = x[c * TPC:(c + 1) * TPC]
        m["x"] = np.ascontiguousarray(_np_bf16(xs.T)).reshape(2, 128, TPC)
        in_maps.append(m)
    flags = (mask_nonzero, qkvb_nonzero, projb_nonzero)
    return in_maps, flags


def kernel(**inputs) -> np.ndarray:
    in_maps, flags = _prep_in_maps(inputs)
    run = _get_runner(*flags)
    out = run(in_maps)                          # (8, TPC, DIM) bf16
    return np.asarray(out, dtype=np.float32).reshape(B_, NTOK, DIM)


# revision 11
# speedup vs baseline: 1.2773x; 1.2773x over previous
"""DPB (dynamic position bias) window attention kernel for Trainium2.

Contract: kernel(**inputs) takes the FULL unsharded inputs (numpy) and
returns the FULL output, running a Bass/Tile kernel over 8 NeuronCores
(pure data parallel over the window-batch dim).

Hardcoded problem shapes:
  x    (3136, 64, 256) f32   -> 392 windows / core
  mask (49, 64, 64) f32      (zeros in practice; general path supported)
  out  (3136, 64, 256) f32

Design (v3):
  - scores computed TRANSPOSED (S^T[k, q], keys on partitions) so the
    attention probabilities are already key-major for the AV matmul —
    no PE transposes of P needed.
  - softmax denominator z comes for free from the AV matmul: V is
    augmented with a ones column (33-wide head blocks), so out[q, 32]
    accumulates sum_k E[k, q].
  - normalization deferred to after AV: O~ and z both land q-major in
    PSUM; one cheap reciprocal over 32 strided columns + one fused
    multiply-evacuate produce the normalized context.
  - O is transposed back to feature-major (8 small PE transposes) for
    the output projection.
  - exp(rpb) is folded in post-exp with a j-replicated bf16 table
    (non-broadcast APs keep the DVE in 2x mode).
  - y is stored to HBM in bf16 (halves output DMA); host upcasts.
"""

import sys

sys.path.insert(0, "/opt/trn_rl_repo")

import numpy as np
import ml_dtypes

import concourse.bass as bass
import concourse.tile as tile
from concourse import bacc, mybir
from concourse.masks import make_identity

BF16 = mybir.dt.bfloat16
F32 = mybir.dt.float32
AF = mybir.ActivationFunctionType
ALU = mybir.AluOpType

# ---- problem constants ----------------------------------------------------
DIM = 256
HEADS = 8
HD = 32
NTOK = 64
NW49 = 49
BATCH = 64
B_ = BATCH * NW49          # 3136
NCORES = 8
WPC = B_ // NCORES         # 392 windows per core
TPC = WPC * NTOK           # 25088 tokens per core
CHW = 8                    # windows per chunk
NCHUNK = WPC // CHW        # 49
SCALE = HD ** -0.5


def _np_bf16(a):
    return np.asarray(a, dtype=ml_dtypes.bfloat16)


# ---- host-side DPB MLP + relative-position tables --------------------------
def _host_rpb(inputs):
    """rpb[h, q, k] = p3[rel_idx[q, k], h] where p3 = DPB MLP(biases)."""
    f = lambda k: np.asarray(inputs[k], np.float32)
    biases = f("biases")            # (225, 2)
    eps = 1e-5

    def ln(x, g, b):
        m = x.mean(-1, keepdims=True)
        v = ((x - m) ** 2).mean(-1, keepdims=True)
        return (x - m) / np.sqrt(v + eps) * g + b

    p = biases @ f("pos_proj_w").T + f("pos_proj_b")
    p = np.maximum(ln(p, f("ln1_g"), f("ln1_b")), 0.0) @ f("fc1_w").T + f("fc1_b")
    p = np.maximum(ln(p, f("ln2_g"), f("ln2_b")), 0.0) @ f("fc2_w").T + f("fc2_b")
    p = np.maximum(ln(p, f("ln3_g"), f("ln3_b")), 0.0) @ f("fc3_w").T + f("fc3_b")
    # p: (225, HEADS)
    rel_idx = np.asarray(inputs["rel_idx"], np.int64)      # (64, 64)
    rpb = p[rel_idx]                                        # (q, k, h)
    return np.transpose(rpb, (2, 0, 1))                     # (h, q, k)


def _host_tables(inputs, mask_nonzero):
    """Fast path: tfast [128, 2048] bf16 with
         tfast[k2, r*512 + j*128 + hh*64 + q] = exp(rpb[hh*4+r, q, k2%64])
       (k duplicated over the two 64-partition window halves, replicated
       over j so the multiply AP is non-broadcast -> DVE 2x mode).
       Mask path: etmask [64, 49*4*128] bf16 with
         etmask[k, (t*4 + r)*128 + hh*64 + q] = exp(rpb + mask[t])."""
    rpb = _host_rpb(inputs)                                 # (h, q, k)
    if not mask_nonzero:
        t = np.empty((64, 2048), np.float32)
        for r in range(4):
            for hh in range(2):
                h = hh * 4 + r
                e = np.exp(rpb[h].T)                        # (k, q)
                for j in range(4):
                    t[:, r * 512 + j * 128 + hh * 64:
                         r * 512 + j * 128 + hh * 64 + 64] = e
        return _np_bf16(np.concatenate([t, t], axis=0)), None
    mask = np.asarray(inputs["mask"], np.float32)           # (49, 64, 64)
    em = np.empty((64, NW49 * 4 * 128), np.float32)
    for t49 in range(NW49):
        for r in range(4):
            for hh in range(2):
                h = hh * 4 + r
                e = np.exp(rpb[h] + mask[t49]).T            # (k, q)
                base = (t49 * 4 + r) * 128 + hh * 64
                em[:, base:base + 64] = e
    return None, _np_bf16(em)


# ---- device kernel builder -------------------------------------------------
def _build(mask_nonzero, qkvb_nonzero, projb_nonzero, nchunk=NCHUNK):
    nc = bacc.Bacc("TRN2", target_bir_lowering=False, debug=False)

    x_d = nc.dram_tensor("x", (2, 128, TPC), BF16, kind="ExternalInput")
    y_d = nc.dram_tensor("y", (TPC, DIM), BF16, kind="ExternalOutput")
    wqk_d = nc.dram_tensor("wqk", (2, 128, 512), BF16, kind="ExternalInput")
    wv_d = nc.dram_tensor("wv", (2, 128, 256), BF16, kind="ExternalInput")
    pw_d = nc.dram_tensor("pw", (2, 128, 256), BF16, kind="ExternalInput")
    vtmpl_d = nc.dram_tensor("vtmpl", (128, 2 * 4 * 264), BF16, kind="ExternalInput")
    if mask_nonzero:
        e_d = nc.dram_tensor("etab", (64, NW49 * 4 * 128), BF16, kind="ExternalInput")
    else:
        e_d = nc.dram_tensor("etab", (128, 2048), BF16, kind="ExternalInput")
    if qkvb_nonzero:
        qkb_d = nc.dram_tensor("qkb", (4, 128), F32, kind="ExternalInput")
        vb_d = nc.dram_tensor("vb", (128, 264), BF16, kind="ExternalInput")
    if projb_nonzero:
        yb_d = nc.dram_tensor("yb", (128, 256), F32, kind="ExternalInput")

    with tile.TileContext(nc) as tc:
        with (
            tc.tile_pool(name="setup", bufs=1) as setup,
            tc.tile_pool(name="xts", bufs=3) as xts,
            tc.tile_pool(name="qks", bufs=3) as qks,
            tc.tile_pool(name="ets", bufs=2) as ets,
            tc.tile_pool(name="avqs", bufs=2) as avqs,
            tc.tile_pool(name="avs", bufs=2) as avs,
            tc.tile_pool(name="ys", bufs=3) as ysp,
            tc.tile_pool(name="dst", bufs=4) as dst,
            tc.tile_pool(name="pp_a", bufs=2, space="PSUM") as pp_a,
            tc.tile_pool(name="pp_sc", bufs=2, space="PSUM") as pp_sc,
            tc.tile_pool(name="pp_av", bufs=1, space="PSUM") as pp_av,
        ):
            # ---- one-time setup ----
            ident = setup.tile([128, 128], BF16)
            make_identity(nc, ident)

            wqk = setup.tile([128, 2, 512], BF16)
            nc.gpsimd.dma_start(
                out=wqk,
                in_=bass.AP(tensor=wqk_d, offset=0,
                            ap=[[512, 128], [128 * 512, 2], [1, 512]]),
            )
            wv = setup.tile([128, 2, 256], BF16)
            nc.gpsimd.dma_start(
                out=wv,
                in_=bass.AP(tensor=wv_d, offset=0,
                            ap=[[256, 128], [128 * 256, 2], [1, 256]]),
            )
            pw = setup.tile([128, 2, 256], BF16)
            nc.gpsimd.dma_start(
                out=pw,
                in_=bass.AP(tensor=pw_d, offset=0,
                            ap=[[256, 128], [128 * 256, 2], [1, 256]]),
            )
            # v_aug: persistent double-buffered (h, 33)-block v staging with
            # pre-seeded ones columns (from the host template)
            v_aug = setup.tile([128, 2, 4, 264], BF16)
            nc.gpsimd.dma_start(out=v_aug, in_=vtmpl_d.ap())
            if mask_nonzero:
                etab = setup.tile([64, NW49 * 4 * 128], BF16)
                nc.gpsimd.dma_start(out=etab, in_=e_d.ap())
            else:
                etab = setup.tile([128, 2048], BF16)
                nc.gpsimd.dma_start(out=etab, in_=e_d.ap())
            if qkvb_nonzero:
                qkb = setup.tile([128, 4], F32)
                nc.gpsimd.dma_start(
                    out=qkb,
                    in_=bass.AP(tensor=qkb_d, offset=0, ap=[[1, 128], [128, 4]]),
                )
                vb = setup.tile([128, 264], BF16)
                nc.gpsimd.dma_start(out=vb, in_=vb_d.ap())
            if projb_nonzero:
                yb = setup.tile([128, 256], F32)
                nc.gpsimd.dma_start(out=yb, in_=yb_d.ap())

            # ---- software-pipelined main loop (8 windows / 512 tok per chunk)
            # PE order per iteration: qkv(c) | AV(c-1) | scores(c) | O^T+proj(c-1)
            # so the PE never idles on the chunk-tail DVE chain (keeps HAM warm).

            def head1(c):
                tok0 = c * 512
                par = c % 2
                st = {}
                xt = xts.tile([128, 2, 512], BF16)
                nc.sync.dma_start(
                    out=xt,
                    in_=bass.AP(tensor=x_d, offset=tok0,
                                ap=[[TPC, 128], [128 * TPC, 2], [1, 512]]),
                )
                # qkT = Wqk.T @ xT : 4 m-tiles (q0 q1 k0 k1)
                qk = qks.tile([128, 4, 512], BF16)
                for m in (0, 2, 1, 3):
                    qkp = pp_a.tile([128, 512], F32, name="qkp", tag="mm")
                    for kk in range(2):
                        nc.tensor.matmul(
                            qkp,
                            wqk[:, kk, 128 * m:128 * (m + 1)],
                            xt[:, kk, :],
                            start=(kk == 0),
                            stop=(kk == 1),
                        )
                    if qkvb_nonzero:
                        if m < 2:
                            nc.scalar.activation(
                                qk[:, m, :], qkp, AF.Copy, bias=qkb[:, m:m + 1]
                            )
                        else:
                            nc.vector.tensor_scalar_add(
                                qk[:, m, :], qkp, qkb[:, m:m + 1]
                            )
                    else:
                        if m < 2:
                            nc.scalar.copy(qk[:, m, :], qkp)
                        else:
                            nc.vector.tensor_copy(qk[:, m, :], qkp)

                # v (token-major) -> v_aug (h, 33)-blocks, ones col persists
                for jp in range(2):
                    vp = pp_a.tile([128, 512], F32, name="vp", tag="mm")
                    for j2 in range(2):
                        j = jp * 2 + j2
                        for kk in range(2):
                            nc.tensor.matmul(
                                vp[:, 256 * j2:256 * (j2 + 1)],
                                xt[:, kk, 128 * j:128 * (j + 1)],
                                wv[:, kk, :],
                                start=(kk == 0),
                                stop=(kk == 1),
                            )
                    vdst = bass.AP(
                        tensor=v_aug.tensor,
                        offset=v_aug.offset + par * (4 * 264) + jp * 2 * 264,
                        ap=[v_aug.ap[0], [264, 2], [33, 8], [1, 32]],
                    )
                    vsrc = vp.rearrange("p (j2 h d) -> p j2 h d", j2=2, h=8)
                    if qkvb_nonzero:
                        vb_ap = bass.AP(
                            tensor=vb.tensor, offset=vb.offset,
                            ap=[vb.ap[0], [0, 2], [33, 8], [1, 32]],
                        )
                        nc.vector.tensor_tensor(vdst, vsrc, vb_ap, ALU.add)
                    else:
                        nc.vector.tensor_copy(vdst, vsrc)
                st["qk"] = qk
                return st

            def head2(c, st, rs):
                qk = st["qk"]
                # scores^T: per-r psum [128 = 2win keys, 512 = (j, hh, q)];
                # r-serial so exp(r) frees its bank during r+1's matmuls
                if "et" in st:
                    et = st["et"]
                else:
                    et = ets.tile([128, 2048], BF16)
                for r in rs:
                    sc = pp_sc.tile([128, 512], F32, name=f"sc{r}", tag="sc")
                    scv = sc.rearrange("p (j hh q) -> p j hh q", j=4, hh=2)
                    for j in range(4):
                        for hh in range(2):
                            for win in range(2):
                                base = 128 * j + 64 * win
                                nc.tensor.matmul(
                                    scv[64 * win:64 * (win + 1), j, hh, :],
                                    qk[32 * r:32 * r + 32, 2 + hh,
                                       base:base + 64],
                                    qk[32 * r:32 * r + 32, hh,
                                       base:base + 64],
                                    tile_position=(32 * r, 64 * win),
                                )
                    nc.scalar.activation(
                        et[:, 512 * r:512 * (r + 1)], sc, AF.Exp
                    )

                # fold exp(rpb [+ mask]) multiplicatively
                if not mask_nonzero:
                    for r in rs:
                        sl = et[:, 512 * r:512 * (r + 1)]
                        nc.vector.tensor_tensor(
                            sl, sl, etab[:, 512 * r:512 * (r + 1)], ALU.mult
                        )
                else:
                    for w in range(CHW):
                        t49 = (c * CHW + w) % NW49
                        j, win = w // 2, w % 2
                        for r in rs:
                            sl = et[64 * win:64 * (win + 1),
                                    512 * r + 128 * j:512 * r + 128 * j + 128]
                            e_ap = bass.AP(
                                tensor=etab.tensor,
                                offset=etab.offset + (t49 * 4 + r) * 128,
                                ap=[etab.ap[0], [1, 128]],
                            )
                            nc.vector.tensor_tensor(sl, sl, e_ap, ALU.mult)
                st["et"] = et

            def tail_a(c, st):
                et = st["et"]
                par = c % 2
                # AV (+z): stationary E^T slices, stream v_aug (33-wide);
                # win innermost alternates PE row groups
                av = pp_av.tile([128, 2048], F32, name="av", tag="av")
                for r in range(4):
                    for j in range(4):
                        for hh in range(2):
                            h8 = hh * 4 + r
                            ebase = 512 * r + 128 * j + 64 * hh
                            for win in range(2):
                                nc.tensor.matmul(
                                    av[64 * win:64 * (win + 1),
                                       j * 512 + 33 * h8:j * 512 + 33 * h8 + 33],
                                    et[64 * win:64 * (win + 1), ebase:ebase + 64],
                                    v_aug[64 * win:64 * (win + 1), par, j,
                                          33 * h8:33 * h8 + 33],
                                    tile_position=(64 * win, 64 * win),
                                )

                # reciprocal of z (strided cols 512j + 33h + 32)
                rz = dst.tile([128, 4, 8], F32)
                z_ap = bass.AP(
                    tensor=av.tensor, offset=av.offset + 32,
                    ap=[av.ap[0], [512, 4], [33, 8]],
                )
                nc.vector.reciprocal_approx_fast(rz, z_ap)

                # normalize + evacuate to compact q-major bf16
                avq = avqs.tile([128, 1024], BF16)
                avq_v = avq.rearrange("p (j h d) -> p j h d", j=4, h=8)
                o_ap = bass.AP(
                    tensor=av.tensor, offset=av.offset,
                    ap=[av.ap[0], [512, 4], [33, 8], [1, 32]],
                )
                rz_ap = bass.AP(
                    tensor=rz.tensor, offset=rz.offset,
                    ap=[rz.ap[0], [8, 4], [1, 8], [0, 32]],
                )
                nc.vector.tensor_tensor(avq_v, o_ap, rz_ap, ALU.mult)
                st["avq"] = avq

            def tail_b(c, st):
                avq = st["avq"]
                tok0 = c * 512
                # transpose O back to feature-major for proj
                av_sb = avs.tile([128, 2, 512], BF16)
                for tp in range(2):
                    tr = pp_a.tile([128, 512], BF16, name=f"tr{tp}", tag="mm")
                    for i in range(4):
                        j = tp * 2 + i // 2
                        kk = i % 2
                        nc.tensor.transpose(
                            tr[:, 128 * i:128 * (i + 1)],
                            avq[:, 256 * j + 128 * kk:256 * j + 128 * kk + 128],
                            ident,
                        )
                    # tr cols = (j2, kk, q) -> av_sb[., kk, 128j + q]
                    tdst = bass.AP(
                        tensor=av_sb.tensor,
                        offset=av_sb.offset + tp * 256,
                        ap=[av_sb.ap[0], [128, 2], [512, 2], [1, 128]],
                    )
                    tsrc = tr.rearrange("p (j2 kk q) -> p j2 kk q", j2=2, kk=2)
                    if tp == 0:
                        nc.vector.tensor_copy(tdst, tsrc)
                    else:
                        nc.scalar.copy(tdst, tsrc)

                # proj
                y_sb = ysp.tile([128, 4, 256], BF16)
                for yp2 in range(2):
                    yp = pp_a.tile([128, 512], F32, name=f"yp{yp2}", tag="mm")
                    for l in range(2):
                        jj = yp2 * 2 + l
                        for kk in range(2):
                            nc.tensor.matmul(
                                yp[:, 256 * l:256 * (l + 1)],
                                av_sb[:, kk, 128 * jj:128 * (jj + 1)],
                                pw[:, kk, :],
                                start=(kk == 0),
                                stop=(kk == 1),
                            )
                    if projb_nonzero:
                        yb_ap = bass.AP(
                            tensor=yb.tensor, offset=yb.offset,
                            ap=[yb.ap[0], [0, 2], [1, 256]],
                        )
                        ydst = y_sb[:, 2 * yp2:2 * yp2 + 2, :]
                        nc.vector.tensor_tensor(
                            ydst, yp.rearrange("p (l f) -> p l f", l=2),
                            yb_ap, ALU.add,
                        )
                    else:
                        nc.scalar.copy(y_sb[:, 2 * yp2:2 * yp2 + 2, :], yp)

                nc.sync.dma_start(
                    out=bass.AP(tensor=y_d, offset=tok0 * DIM,
                                ap=[[DIM, 128], [128 * DIM, 4], [1, DIM]]),
                    in_=y_sb,
                )

            states = {0: head1(0)}
            head2(0, states[0], (0, 1))
            head2(0, states[0], (2, 3))
            for c in range(1, nchunk):
                states[c] = head1(c)
                tail_a(c - 1, states[c - 1])
                head2(c, states[c], (0, 1))
                tail_b(c - 1, states[c - 1])
                head2(c, states[c], (2, 3))
                del states[c - 1]
            tail_a(nchunk - 1, states[nchunk - 1])
            tail_b(nchunk - 1, states[nchunk - 1])

    nc.compile()
    return nc


# ---- execution --------------------------------------------------------------
_CACHE = {}


def _get_runner(mask_nonzero, qkvb_nonzero, projb_nonzero, nchunk=NCHUNK):
    key = (mask_nonzero, qkvb_nonzero, projb_nonzero, nchunk)
    if key in _CACHE:
        return _CACHE[key]

    nc = _build(mask_nonzero, qkvb_nonzero, projb_nonzero, nchunk)

    import jax
    import jax.numpy as jnp
    from jax.sharding import Mesh, PartitionSpec
    from jax.experimental.shard_map import shard_map
    from concourse import bass2jax
    from concourse.bass2jax import _bass_exec_p, install_neuronx_cc_hook

    install_neuronx_cc_hook()

    partition_name = (
        nc.partition_id_tensor.name if nc.partition_id_tensor else None
    )
    in_names, out_names, out_avals, zero_outs = [], [], [], []
    for alloc in nc.m.functions[0].allocations:
        if not isinstance(alloc, mybir.MemoryLocationSet):
            continue
        name = alloc.memorylocations[0].name
        if alloc.kind == "ExternalInput":
            if name != partition_name:
                in_names.append(name)
        elif alloc.kind == "ExternalOutput":
            shape = tuple(alloc.tensor_shape)
            dtype = mybir.dt.np(alloc.dtype)
            out_names.append(name)
            out_avals.append(jax.core.ShapedArray(shape, dtype))
            zero_outs.append(np.zeros(shape, dtype))
    n_params = len(in_names)
    n_outs = len(out_avals)
    all_in_names = list(in_names) + list(out_names)
    if partition_name is not None:
        all_in_names.append(partition_name)

    def _body(*args):
        operands = list(args)
        if partition_name is not None:
            operands.append(bass2jax.partition_id_tensor())
        outs = _bass_exec_p.bind(
            *operands,
            out_avals=tuple(out_avals),
            in_names=tuple(all_in_names),
            out_names=tuple(out_names),
            lowering_input_output_aliases=(),
            sim_require_finite=True,
            sim_require_nnan=True,
            nc=nc,
        )
        return tuple(outs)

    devices = jax.devices()[:NCORES]
    mesh = Mesh(np.asarray(devices), ("core",))
    donate = tuple(range(n_params, n_params + n_outs))
    sharded = jax.jit(
        shard_map(
            _body, mesh=mesh,
            in_specs=(PartitionSpec("core"),) * (n_params + n_outs),
            out_specs=(PartitionSpec("core"),) * n_outs,
            check_rep=False,
        ),
        donate_argnums=donate,
        keep_unused=True,
    )

    from jax.sharding import NamedSharding

    shard = NamedSharding(mesh, PartitionSpec("core"))
    zero_shapes = [
        ((NCORES * z.shape[0], *z.shape[1:]), z.dtype) for z in zero_outs
    ]
    make_zeros = jax.jit(
        lambda: tuple(jnp.zeros(s, d) for s, d in zero_shapes),
        out_shardings=(shard,) * n_outs,
    )

    def _concat(in_maps):
        return [
            np.concatenate([np.asarray(in_maps[c][nm]) for c in range(NCORES)], axis=0)
            for nm in in_names
        ]

    def run(in_maps):
        out_arrs = sharded(*_concat(in_maps), *make_zeros())
        out = np.asarray(out_arrs[out_names.index("y")])
        return out.reshape(NCORES, TPC, DIM)

    def bench(in_maps, iters=8):
        import time as _time

        dev_in = [jax.device_put(a, shard) for a in _concat(in_maps)]
        jax.block_until_ready(dev_in)
        outs = sharded(*dev_in, *make_zeros())
        jax.block_until_ready(outs)
        ts = []
        for _ in range(iters):
            t0 = _time.perf_counter()
            outs = sharded(*dev_in, *make_zeros())
            jax.block_until_ready(outs)
            ts.append(_time.perf_counter() - t0)
        return min(ts), ts

    def bench_repeat(in_maps, reps, iters=6):
        """Async-chain `reps` dispatches (output ping-pongs into the donated
        slot) and block once; median over iters."""
        import time as _time

        dev_in = [jax.device_put(a, shard) for a in _concat(in_maps)]
        jax.block_until_ready(dev_in)
        outs = sharded(*dev_in, *make_zeros())
        jax.block_until_ready(outs)
        ts = []
        for _ in range(iters):
            outs = sharded(*dev_in, *make_zeros())
            jax.block_until_ready(outs)
            t0 = _time.perf_counter()
            for _ in range(reps):
                outs = sharded(*dev_in, *outs)
            jax.block_until_ready(outs)
            ts.append(_time.perf_counter() - t0)
        ts.sort()
        return ts[len(ts) // 2], ts

    run.bench = bench
    run.bench_repeat = bench_repeat
    _CACHE[key] = run
    return run


def _prep_in_maps(inputs):
    x = np.asarray(inputs["x"], np.float32).reshape(B_ * NTOK, DIM)
    qkv_w = np.asarray(inputs["qkv_w"], np.float32)
    qkv_b = np.asarray(inputs["qkv_b"], np.float32)
    proj_w = np.asarray(inputs["proj_w"], np.float32)
    proj_b = np.asarray(inputs["proj_b"], np.float32)
    mask_nonzero = bool(np.any(np.asarray(inputs["mask"]) != 0))
    qkvb_nonzero = bool(np.any(qkv_b != 0))
    projb_nonzero = bool(np.any(proj_b != 0))

    wqk_f = qkv_w[:512].copy()
    wqk_f[:256] *= SCALE                       # fold q scale into Wq
    wqk = _np_bf16(wqk_f.T.reshape(2, 128, 512))
    wv = _np_bf16(qkv_w[512:].T.reshape(2, 128, 256))
    pw = _np_bf16(proj_w.T.reshape(2, 128, 256))

    tfast, etmask = _host_tables(inputs, mask_nonzero)
    etab = etmask if mask_nonzero else tfast

    # v_aug template: zeros with ones in the 33rd column of each head block
    vtmpl = np.zeros((128, 2, 4, 8, 33), np.float32)
    vtmpl[:, :, :, :, 32] = 1.0
    vtmpl = _np_bf16(vtmpl.reshape(128, 2 * 4 * 264))

    shared = {"wqk": wqk, "wv": wv, "pw": pw, "etab": etab, "vtmpl": vtmpl}
    if qkvb_nonzero:
        qkb_f = qkv_b[:512].copy()
        qkb_f[:256] *= SCALE
        shared["qkb"] = qkb_f.reshape(4, 128).astype(np.float32)
        vb_aug = np.zeros((128, 8, 33), np.float32)
        vb_aug[:, :, :32] = qkv_b[512:].reshape(1, 8, 32)
        shared["vb"] = _np_bf16(vb_aug.reshape(128, 264))
    if projb_nonzero:
        shared["yb"] = np.broadcast_to(proj_b, (128, 256)).copy().astype(np.float32)

    in_maps = []
    for c in range(NCORES):
        m = dict(shared)
        xs = x[c * TPC:(c + 1) * TPC]
        m["x"] = np.ascontiguousarray(_np_bf16(xs.T)).reshape(2, 128, TPC)
        in_maps.append(m)
    flags = (mask_nonzero, qkvb_nonzero, projb_nonzero)
    return in_maps, flags


def kernel(**inputs) -> np.ndarray:
    in_maps, flags = _prep_in_maps(inputs)
    run = _get_runner(*flags)
    out = run(in_maps)                          # (8, TPC, DIM) bf16
    return np.asarray(out, dtype=np.float32).reshape(B_, NTOK, DIM)


# revision 14
# speedup vs baseline: 2.4799x; 1.9415x over previous
"""DPB (dynamic position bias) window attention kernel for Trainium2.

Contract: kernel(**inputs) takes the FULL unsharded inputs (numpy) and
returns the FULL output, running a Bass/Tile kernel over 8 NeuronCores
(pure data parallel over the window-batch dim).

Hardcoded problem shapes:
  x    (3136, 64, 256) f32   -> 392 windows / core
  mask (49, 64, 64) f32      (zeros in practice; general path supported)
  out  (3136, 64, 256) f32

Design (v3):
  - scores computed TRANSPOSED (S^T[k, q], keys on partitions) so the
    attention probabilities are already key-major for the AV matmul —
    no PE transposes of P needed.
  - softmax denominator z comes for free from the AV matmul: V is
    augmented with a ones column (33-wide head blocks), so out[q, 32]
    accumulates sum_k E[k, q].
  - normalization deferred to after AV: O~ and z both land q-major in
    PSUM; one cheap reciprocal over 32 strided columns + one fused
    multiply-evacuate produce the normalized context.
  - O is transposed back to feature-major (8 small PE transposes) for
    the output projection.
  - exp(rpb) is folded in post-exp with a j-replicated bf16 table
    (non-broadcast APs keep the DVE in 2x mode).
  - y is stored to HBM in bf16 (halves output DMA); host upcasts.
"""

import sys

sys.path.insert(0, "/opt/trn_rl_repo")

import numpy as np
import ml_dtypes

import concourse.bass as bass
import concourse.tile as tile
from concourse import bacc, mybir
from concourse.masks import make_identity

BF16 = mybir.dt.bfloat16
F32 = mybir.dt.float32
AF = mybir.ActivationFunctionType
ALU = mybir.AluOpType

# ---- problem constants ----------------------------------------------------
DIM = 256
HEADS = 8
HD = 32
NTOK = 64
NW49 = 49
BATCH = 64
B_ = BATCH * NW49          # 3136
NCORES = 8
WPC = B_ // NCORES         # 392 windows per core
TPC = WPC * NTOK           # 25088 tokens per core
CHW = 8                    # windows per chunk
NCHUNK = WPC // CHW        # 49
SCALE = HD ** -0.5


def _np_bf16(a):
    return np.asarray(a, dtype=ml_dtypes.bfloat16)


# ---- host-side DPB MLP + relative-position tables --------------------------
def _host_rpb(inputs):
    """rpb[h, q, k] = p3[rel_idx[q, k], h] where p3 = DPB MLP(biases)."""
    f = lambda k: np.asarray(inputs[k], np.float32)
    biases = f("biases")            # (225, 2)
    eps = 1e-5

    def ln(x, g, b):
        m = x.mean(-1, keepdims=True)
        v = ((x - m) ** 2).mean(-1, keepdims=True)
        return (x - m) / np.sqrt(v + eps) * g + b

    p = biases @ f("pos_proj_w").T + f("pos_proj_b")
    p = np.maximum(ln(p, f("ln1_g"), f("ln1_b")), 0.0) @ f("fc1_w").T + f("fc1_b")
    p = np.maximum(ln(p, f("ln2_g"), f("ln2_b")), 0.0) @ f("fc2_w").T + f("fc2_b")
    p = np.maximum(ln(p, f("ln3_g"), f("ln3_b")), 0.0) @ f("fc3_w").T + f("fc3_b")
    # p: (225, HEADS)
    rel_idx = np.asarray(inputs["rel_idx"], np.int64)      # (64, 64)
    rpb = p[rel_idx]                                        # (q, k, h)
    return np.transpose(rpb, (2, 0, 1))                     # (h, q, k)


def _host_tables(inputs, mask_nonzero):
    """Fast path: tfast [128, 2048] bf16 with
         tfast[k2, r*512 + j*128 + hh*64 + q] = exp(rpb[hh*4+r, q, k2%64])
       (k duplicated over the two 64-partition window halves, replicated
       over j so the multiply AP is non-broadcast -> DVE 2x mode).
       Mask path: etmask [64, 49*4*128] bf16 with
         etmask[k, (t*4 + r)*128 + hh*64 + q] = exp(rpb + mask[t])."""
    rpb = _host_rpb(inputs)                                 # (h, q, k)
    if not mask_nonzero:
        t = np.empty((64, 2048), np.float32)
        for r in range(4):
            for hh in range(2):
                h = hh * 4 + r
                e = np.exp(rpb[h].T)                        # (k, q)
                for j in range(4):
                    t[:, r * 512 + j * 128 + hh * 64:
                         r * 512 + j * 128 + hh * 64 + 64] = e
        return _np_bf16(np.concatenate([t, t], axis=0)), None
    mask = np.asarray(inputs["mask"], np.float32)           # (49, 64, 64)
    em = np.empty((64, NW49 * 4 * 128), np.float32)
    for t49 in range(NW49):
        for r in range(4):
            for hh in range(2):
                h = hh * 4 + r
                e = np.exp(rpb[h] + mask[t49]).T            # (k, q)
                base = (t49 * 4 + r) * 128 + hh * 64
                em[:, base:base + 64] = e
    return None, _np_bf16(em)


# ---- device kernel builder -------------------------------------------------
def _build(mask_nonzero, qkvb_nonzero, projb_nonzero, nchunk=NCHUNK):
    nc = bacc.Bacc("TRN2", target_bir_lowering=False, debug=False)

    x_d = nc.dram_tensor("x", (2, 128, TPC), BF16, kind="ExternalInput")
    y_d = nc.dram_tensor("y", (TPC, DIM), BF16, kind="ExternalOutput")
    wqk_d = nc.dram_tensor("wqk", (2, 128, 512), BF16, kind="ExternalInput")
    wv_d = nc.dram_tensor("wv", (2, 128, 256), BF16, kind="ExternalInput")
    pw_d = nc.dram_tensor("pw", (2, 128, 256), BF16, kind="ExternalInput")
    vtmpl_d = nc.dram_tensor("vtmpl", (128, 2 * 4 * 264), BF16, kind="ExternalInput")
    if mask_nonzero:
        e_d = nc.dram_tensor("etab", (64, NW49 * 4 * 128), BF16, kind="ExternalInput")
    else:
        e_d = nc.dram_tensor("etab", (128, 2048), BF16, kind="ExternalInput")
    if qkvb_nonzero:
        qkb_d = nc.dram_tensor("qkb", (4, 128), F32, kind="ExternalInput")
        vb_d = nc.dram_tensor("vb", (128, 264), BF16, kind="ExternalInput")
    if projb_nonzero:
        yb_d = nc.dram_tensor("yb", (128, 256), F32, kind="ExternalInput")

    with tile.TileContext(nc) as tc:
        with (
            tc.tile_pool(name="setup", bufs=1) as setup,
            tc.tile_pool(name="xts", bufs=3) as xts,
            tc.tile_pool(name="qks", bufs=3) as qks,
            tc.tile_pool(name="ets", bufs=2) as ets,
            tc.tile_pool(name="avqs", bufs=2) as avqs,
            tc.tile_pool(name="avs", bufs=2) as avs,
            tc.tile_pool(name="ys", bufs=3) as ysp,
            tc.tile_pool(name="dst", bufs=4) as dst,
            tc.tile_pool(name="pp_a", bufs=2, space="PSUM") as pp_a,
            tc.tile_pool(name="pp_sc", bufs=2, space="PSUM") as pp_sc,
            tc.tile_pool(name="pp_av", bufs=1, space="PSUM") as pp_av,
        ):
            # ---- one-time setup ----
            ident = setup.tile([128, 128], BF16)
            make_identity(nc, ident)

            wqk = setup.tile([128, 2, 512], BF16)
            nc.gpsimd.dma_start(
                out=wqk,
                in_=bass.AP(tensor=wqk_d, offset=0,
                            ap=[[512, 128], [128 * 512, 2], [1, 512]]),
            )
            wv = setup.tile([128, 2, 256], BF16)
            nc.gpsimd.dma_start(
                out=wv,
                in_=bass.AP(tensor=wv_d, offset=0,
                            ap=[[256, 128], [128 * 256, 2], [1, 256]]),
            )
            pw = setup.tile([128, 2, 256], BF16)
            nc.gpsimd.dma_start(
                out=pw,
                in_=bass.AP(tensor=pw_d, offset=0,
                            ap=[[256, 128], [128 * 256, 2], [1, 256]]),
            )
            # v_aug: persistent double-buffered (h, 33)-block v staging with
            # pre-seeded ones columns (from the host template)
            v_aug = setup.tile([128, 2, 4, 264], BF16)
            nc.gpsimd.dma_start(out=v_aug, in_=vtmpl_d.ap())
            if mask_nonzero:
                etab = setup.tile([64, NW49 * 4 * 128], BF16)
                nc.gpsimd.dma_start(out=etab, in_=e_d.ap())
            else:
                etab = setup.tile([128, 2048], BF16)
                nc.gpsimd.dma_start(out=etab, in_=e_d.ap())
            if qkvb_nonzero:
                qkb = setup.tile([128, 4], F32)
                nc.gpsimd.dma_start(
                    out=qkb,
                    in_=bass.AP(tensor=qkb_d, offset=0, ap=[[1, 128], [128, 4]]),
                )
                vb = setup.tile([128, 264], BF16)
                nc.gpsimd.dma_start(out=vb, in_=vb_d.ap())
            if projb_nonzero:
                yb = setup.tile([128, 256], F32)
                nc.gpsimd.dma_start(out=yb, in_=yb_d.ap())

            # ---- software-pipelined main loop (8 windows / 512 tok per chunk)
            # PE order per iteration: qkv(c) | AV(c-1) | scores(c) | O^T+proj(c-1)
            # so the PE never idles on the chunk-tail DVE chain (keeps HAM warm).

            def head1(c):
                tok0 = c * 512
                par = c % 2
                st = {}
                xt = xts.tile([128, 2, 512], BF16)
                nc.sync.dma_start(
                    out=xt,
                    in_=bass.AP(tensor=x_d, offset=tok0,
                                ap=[[TPC, 128], [128 * TPC, 2], [1, 512]]),
                )
                # qkT = Wqk.T @ xT : 4 m-tiles (q0 q1 k0 k1)
                qk = qks.tile([128, 4, 512], BF16)
                for m in (0, 2, 1, 3):
                    qkp = pp_a.tile([128, 512], F32, name="qkp", tag="mm")
                    for kk in range(2):
                        nc.tensor.matmul(
                            qkp,
                            wqk[:, kk, 128 * m:128 * (m + 1)],
                            xt[:, kk, :],
                            start=(kk == 0),
                            stop=(kk == 1),
                        )
                    if qkvb_nonzero:
                        if m < 2:
                            nc.scalar.activation(
                                qk[:, m, :], qkp, AF.Copy, bias=qkb[:, m:m + 1]
                            )
                        else:
                            nc.vector.tensor_scalar_add(
                                qk[:, m, :], qkp, qkb[:, m:m + 1]
                            )
                    else:
                        if m < 2:
                            nc.scalar.copy(qk[:, m, :], qkp)
                        else:
                            nc.vector.tensor_copy(qk[:, m, :], qkp)

                # v (token-major) -> v_aug (h, 33)-blocks, ones col persists
                for jp in range(2):
                    vp = pp_a.tile([128, 512], F32, name="vp", tag="mm")
                    for j2 in range(2):
                        j = jp * 2 + j2
                        for kk in range(2):
                            nc.tensor.matmul(
                                vp[:, 256 * j2:256 * (j2 + 1)],
                                xt[:, kk, 128 * j:128 * (j + 1)],
                                wv[:, kk, :],
                                start=(kk == 0),
                                stop=(kk == 1),
                            )
                    vdst = bass.AP(
                        tensor=v_aug.tensor,
                        offset=v_aug.offset + par * (4 * 264) + jp * 2 * 264,
                        ap=[v_aug.ap[0], [264, 2], [33, 8], [1, 32]],
                    )
                    vsrc = vp.rearrange("p (j2 h d) -> p j2 h d", j2=2, h=8)
                    if qkvb_nonzero:
                        vb_ap = bass.AP(
                            tensor=vb.tensor, offset=vb.offset,
                            ap=[vb.ap[0], [0, 2], [33, 8], [1, 32]],
                        )
                        nc.vector.tensor_tensor(vdst, vsrc, vb_ap, ALU.add)
                    else:
                        nc.vector.tensor_copy(vdst, vsrc)
                st["qk"] = qk
                return st

            def head2(c, st, rs):
                qk = st["qk"]
                # scores^T: per-r psum [128 = 2win keys, 512 = (j, hh, q)];
                # r-serial so exp(r) frees its bank during r+1's matmuls
                if "et" in st:
                    et = st["et"]
                else:
                    et = ets.tile([128, 2048], BF16)
                for r in rs:
                    sc = pp_sc.tile([128, 512], F32, name=f"sc{r}", tag="sc")
                    scv = sc.rearrange("p (j hh q) -> p j hh q", j=4, hh=2)
                    for j in range(4):
                        for hh in range(2):
                            for win in range(2):
                                base = 128 * j + 64 * win
                                nc.tensor.matmul(
                                    scv[64 * win:64 * (win + 1), j, hh, :],
                                    qk[32 * r:32 * r + 32, 2 + hh,
                                       base:base + 64],
                                    qk[32 * r:32 * r + 32, hh,
                                       base:base + 64],
                                    tile_position=(32 * r, 64 * win),
                                )
                    nc.scalar.activation(
                        et[:, 512 * r:512 * (r + 1)], sc, AF.Exp
                    )

                # fold exp(rpb [+ mask]) multiplicatively
                if not mask_nonzero:
                    for r in rs:
                        sl = et[:, 512 * r:512 * (r + 1)]
                        nc.vector.tensor_tensor(
                            sl, sl, etab[:, 512 * r:512 * (r + 1)], ALU.mult
                        )
                else:
                    for w in range(CHW):
                        t49 = (c * CHW + w) % NW49
                        j, win = w // 2, w % 2
                        for r in rs:
                            sl = et[64 * win:64 * (win + 1),
                                    512 * r + 128 * j:512 * r + 128 * j + 128]
                            e_ap = bass.AP(
                                tensor=etab.tensor,
                                offset=etab.offset + (t49 * 4 + r) * 128,
                                ap=[etab.ap[0], [1, 128]],
                            )
                            nc.vector.tensor_tensor(sl, sl, e_ap, ALU.mult)
                st["et"] = et

            def tail_a(c, st):
                et = st["et"]
                par = c % 2
                # AV (+z): stationary E^T slices, stream v_aug (33-wide);
                # win innermost alternates PE row groups
                av = pp_av.tile([128, 2048], F32, name="av", tag="av")
                for r in range(4):
                    for j in range(4):
                        for hh in range(2):
                            h8 = hh * 4 + r
                            ebase = 512 * r + 128 * j + 64 * hh
                            for win in range(2):
                                nc.tensor.matmul(
                                    av[64 * win:64 * (win + 1),
                                       j * 512 + 33 * h8:j * 512 + 33 * h8 + 33],
                                    et[64 * win:64 * (win + 1), ebase:ebase + 64],
                                    v_aug[64 * win:64 * (win + 1), par, j,
                                          33 * h8:33 * h8 + 33],
                                    tile_position=(64 * win, 64 * win),
                                )

                # reciprocal of z (strided cols 512j + 33h + 32)
                rz = dst.tile([128, 4, 8], F32)
                z_ap = bass.AP(
                    tensor=av.tensor, offset=av.offset + 32,
                    ap=[av.ap[0], [512, 4], [33, 8]],
                )
                nc.vector.reciprocal_approx_fast(rz, z_ap)

                # normalize + evacuate to compact q-major bf16
                avq = avqs.tile([128, 1024], BF16)
                avq_v = avq.rearrange("p (j h d) -> p j h d", j=4, h=8)
                o_ap = bass.AP(
                    tensor=av.tensor, offset=av.offset,
                    ap=[av.ap[0], [512, 4], [33, 8], [1, 32]],
                )
                rz_ap = bass.AP(
                    tensor=rz.tensor, offset=rz.offset,
                    ap=[rz.ap[0], [8, 4], [1, 8], [0, 32]],
                )
                nc.vector.tensor_tensor(avq_v, o_ap, rz_ap, ALU.mult)
                st["avq"] = avq

            def tail_b1(c, st):
                avq = st["avq"]
                # transpose O back to feature-major for proj
                av_sb = avs.tile([128, 2, 512], BF16)
                for tp in range(2):
                    tr = pp_a.tile([128, 512], BF16, name=f"tr{tp}", tag="mm")
                    for i in range(4):
                        j = tp * 2 + i // 2
                        kk = i % 2
                        nc.tensor.transpose(
                            tr[:, 128 * i:128 * (i + 1)],
                            avq[:, 256 * j + 128 * kk:256 * j + 128 * kk + 128],
                            ident,
                        )
                    # tr cols = (j2, kk, q) -> av_sb[., kk, 128j + q]
                    tdst = bass.AP(
                        tensor=av_sb.tensor,
                        offset=av_sb.offset + tp * 256,
                        ap=[av_sb.ap[0], [128, 2], [512, 2], [1, 128]],
                    )
                    tsrc = tr.rearrange("p (j2 kk q) -> p j2 kk q", j2=2, kk=2)
                    if tp == 0:
                        nc.vector.tensor_copy(tdst, tsrc)
                    else:
                        nc.scalar.copy(tdst, tsrc)
                st["av_sb"] = av_sb

            def tail_b2(c, st):
                av_sb = st["av_sb"]
                tok0 = c * 512
                # proj
                y_sb = ysp.tile([128, 4, 256], BF16)
                for yp2 in range(2):
                    yp = pp_a.tile([128, 512], F32, name=f"yp{yp2}", tag="mm")
                    for l in range(2):
                        jj = yp2 * 2 + l
                        for kk in range(2):
                            nc.tensor.matmul(
                                yp[:, 256 * l:256 * (l + 1)],
                                av_sb[:, kk, 128 * jj:128 * (jj + 1)],
                                pw[:, kk, :],
                                start=(kk == 0),
                                stop=(kk == 1),
                            )
                    if projb_nonzero:
                        yb_ap = bass.AP(
                            tensor=yb.tensor, offset=yb.offset,
                            ap=[yb.ap[0], [0, 2], [1, 256]],
                        )
                        ydst = y_sb[:, 2 * yp2:2 * yp2 + 2, :]
                        nc.vector.tensor_tensor(
                            ydst, yp.rearrange("p (l f) -> p l f", l=2),
                            yb_ap, ALU.add,
                        )
                    else:
                        nc.scalar.copy(y_sb[:, 2 * yp2:2 * yp2 + 2, :], yp)

                nc.sync.dma_start(
                    out=bass.AP(tensor=y_d, offset=tok0 * DIM,
                                ap=[[DIM, 128], [128 * DIM, 4], [1, DIM]]),
                    in_=y_sb,
                )

            states = {0: head1(0)}
            head2(0, states[0], (0, 1))
            head2(0, states[0], (2, 3))
            for c in range(1, nchunk):
                states[c] = head1(c)
                tail_a(c - 1, states[c - 1])
                head2(c, states[c], (0, 1))
                tail_b1(c - 1, states[c - 1])
                head2(c, states[c], (2, 3))
                tail_b2(c - 1, states[c - 1])
                del states[c - 1]
            tail_a(nchunk - 1, states[nchunk - 1])
            tail_b1(nchunk - 1, states[nchunk - 1])
            tail_b2(nchunk - 1, states[nchunk - 1])

    nc.compile()
    return nc


# ---- execution --------------------------------------------------------------
_CACHE = {}


def _get_runner(mask_nonzero, qkvb_nonzero, projb_nonzero, nchunk=NCHUNK):
    key = (mask_nonzero, qkvb_nonzero, projb_nonzero, nchunk)
    if key in _CACHE:
        return _CACHE[key]

    nc = _build(mask_nonzero, qkvb_nonzero, projb_nonzero, nchunk)

    import jax
    import jax.numpy as jnp
    from jax.sharding import Mesh, PartitionSpec
    from jax.experimental.shard_map import shard_map
    from concourse import bass2jax
    from concourse.bass2jax import _bass_exec_p, install_neuronx_cc_hook

    install_neuronx_cc_hook()

    partition_name = (
        nc.partition_id_tensor.name if nc.partition_id_tensor else None
    )
    in_names, out_names, out_avals, zero_outs = [], [], [], []
    for alloc in nc.m.functions[0].allocations:
        if not isinstance(alloc, mybir.MemoryLocationSet):
            continue
        name = alloc.memorylocations[0].name
        if alloc.kind == "ExternalInput":
            if name != partition_name:
                in_names.append(name)
        elif alloc.kind == "ExternalOutput":
            shape = tuple(alloc.tensor_shape)
            dtype = mybir.dt.np(alloc.dtype)
            out_names.append(name)
            out_avals.append(jax.core.ShapedArray(shape, dtype))
            zero_outs.append(np.zeros(shape, dtype))
    n_params = len(in_names)
    n_outs = len(out_avals)
    all_in_names = list(in_names) + list(out_names)
    if partition_name is not None:
        all_in_names.append(partition_name)

    def _body(*args):
        operands = list(args)
        if partition_name is not None:
            operands.append(bass2jax.partition_id_tensor())
        outs = _bass_exec_p.bind(
            *operands,
            out_avals=tuple(out_avals),
            in_names=tuple(all_in_names),
            out_names=tuple(out_names),
            lowering_input_output_aliases=(),
            sim_require_finite=True,
            sim_require_nnan=True,
            nc=nc,
        )
        return tuple(outs)

    devices = jax.devices()[:NCORES]
    mesh = Mesh(np.asarray(devices), ("core",))
    donate = tuple(range(n_params, n_params + n_outs))
    sharded = jax.jit(
        shard_map(
            _body, mesh=mesh,
            in_specs=(PartitionSpec("core"),) * (n_params + n_outs),
            out_specs=(PartitionSpec("core"),) * n_outs,
            check_rep=False,
        ),
        donate_argnums=donate,
        keep_unused=True,
    )

    from jax.sharding import NamedSharding

    shard = NamedSharding(mesh, PartitionSpec("core"))
    zero_shapes = [
        ((NCORES * z.shape[0], *z.shape[1:]), z.dtype) for z in zero_outs
    ]
    make_zeros = jax.jit(
        lambda: tuple(jnp.zeros(s, d) for s, d in zero_shapes),
        out_shardings=(shard,) * n_outs,
    )

    def _concat(in_maps):
        return [
            np.concatenate([np.asarray(in_maps[c][nm]) for c in range(NCORES)], axis=0)
            for nm in in_names
        ]

    def run(in_maps):
        out_arrs = sharded(*_concat(in_maps), *make_zeros())
        out = np.asarray(out_arrs[out_names.index("y")])
        return out.reshape(NCORES, TPC, DIM)

    def bench(in_maps, iters=8):
        import time as _time

        dev_in = [jax.device_put(a, shard) for a in _concat(in_maps)]
        jax.block_until_ready(dev_in)
        outs = sharded(*dev_in, *make_zeros())
        jax.block_until_ready(outs)
        ts = []
        for _ in range(iters):
            t0 = _time.perf_counter()
            outs = sharded(*dev_in, *make_zeros())
            jax.block_until_ready(outs)
            ts.append(_time.perf_counter() - t0)
        return min(ts), ts

    def bench_repeat(in_maps, reps, iters=6):
        """Async-chain `reps` dispatches (output ping-pongs into the donated
        slot) and block once; median over iters."""
        import time as _time

        dev_in = [jax.device_put(a, shard) for a in _concat(in_maps)]
        jax.block_until_ready(dev_in)
        outs = sharded(*dev_in, *make_zeros())
        jax.block_until_ready(outs)
        ts = []
        for _ in range(iters):
            outs = sharded(*dev_in, *make_zeros())
            jax.block_until_ready(outs)
            t0 = _time.perf_counter()
            for _ in range(reps):
                outs = sharded(*dev_in, *outs)
            jax.block_until_ready(outs)
            ts.append(_time.perf_counter() - t0)
        ts.sort()
        return ts[len(ts) // 2], ts

    run.bench = bench
    run.bench_repeat = bench_repeat
    _CACHE[key] = run
    return run


def _prep_in_maps(inputs):
    x = np.asarray(inputs["x"], np.float32).reshape(B_ * NTOK, DIM)
    qkv_w = np.asarray(inputs["qkv_w"], np.float32)
    qkv_b = np.asarray(inputs["qkv_b"], np.float32)
    proj_w = np.asarray(inputs["proj_w"], np.float32)
    proj_b = np.asarray(inputs["proj_b"], np.float32)
    mask_nonzero = bool(np.any(np.asarray(inputs["mask"]) != 0))
    qkvb_nonzero = bool(np.any(qkv_b != 0))
    projb_nonzero = bool(np.any(proj_b != 0))

    wqk_f = qkv_w[:512].copy()
    wqk_f[:256] *= SCALE                       # fold q scale into Wq
    wqk = _np_bf16(wqk_f.T.reshape(2, 128, 512))
    wv = _np_bf16(qkv_w[512:].T.reshape(2, 128, 256))
    pw = _np_bf16(proj_w.T.reshape(2, 128, 256))

    tfast, etmask = _host_tables(inputs, mask_nonzero)
    etab = etmask if mask_nonzero else tfast

    # v_aug template: zeros with ones in the 33rd column of each head block
    vtmpl = np.zeros((128, 2, 4, 8, 33), np.float32)
    vtmpl[:, :, :, :, 32] = 1.0
    vtmpl = _np_bf16(vtmpl.reshape(128, 2 * 4 * 264))

    shared = {"wqk": wqk, "wv": wv, "pw": pw, "etab": etab, "vtmpl": vtmpl}
    if qkvb_nonzero:
        qkb_f = qkv_b[:512].copy()
        qkb_f[:256] *= SCALE
        shared["qkb"] = qkb_f.reshape(4, 128).astype(np.float32)
        vb_aug = np.zeros((128, 8, 33), np.float32)
        vb_aug[:, :, :32] = qkv_b[512:].reshape(1, 8, 32)
        shared["vb"] = _np_bf16(vb_aug.reshape(128, 264))
    if projb_nonzero:
        shared["yb"] = np.broadcast_to(proj_b, (128, 256)).copy().astype(np.float32)

    in_maps = []
    for c in range(NCORES):
        m = dict(shared)
        xs = x[c * TPC:(c + 1) * TPC]
        m["x"] = np.ascontiguousarray(_np_bf16(xs.T)).reshape(2, 128, TPC)
        in_maps.append(m)
    flags = (mask_nonzero, qkvb_nonzero, projb_nonzero)
    return in_maps, flags


def kernel(**inputs) -> np.ndarray:
    in_maps, flags = _prep_in_maps(inputs)
    run = _get_runner(*flags)
    out = run(in_maps)                          # (8, TPC, DIM) bf16
    return np.asarray(out, dtype=np.float32).reshape(B_, NTOK, DIM)


# revision 15
# speedup vs baseline: 2.4986x; 1.0075x over previous
"""DPB (dynamic position bias) window attention kernel for Trainium2.

Contract: kernel(**inputs) takes the FULL unsharded inputs (numpy) and
returns the FULL output, running a Bass/Tile kernel over 8 NeuronCores
(pure data parallel over the window-batch dim).

Hardcoded problem shapes:
  x    (3136, 64, 256) f32   -> 392 windows / core
  mask (49, 64, 64) f32      (zeros in practice; general path supported)
  out  (3136, 64, 256) f32

Design (v3):
  - scores computed TRANSPOSED (S^T[k, q], keys on partitions) so the
    attention probabilities are already key-major for the AV matmul —
    no PE transposes of P needed.
  - softmax denominator z comes for free from the AV matmul: V is
    augmented with a ones column (33-wide head blocks), so out[q, 32]
    accumulates sum_k E[k, q].
  - normalization deferred to after AV: O~ and z both land q-major in
    PSUM; one cheap reciprocal over 32 strided columns + one fused
    multiply-evacuate produce the normalized context.
  - O is transposed back to feature-major (8 small PE transposes) for
    the output projection.
  - exp(rpb) is folded in post-exp with a j-replicated bf16 table
    (non-broadcast APs keep the DVE in 2x mode).
  - y is stored to HBM in bf16 (halves output DMA); host upcasts.
"""

import sys

sys.path.insert(0, "/opt/trn_rl_repo")

import numpy as np
import ml_dtypes

import concourse.bass as bass
import concourse.tile as tile
from concourse import bacc, mybir
from concourse.masks import make_identity

BF16 = mybir.dt.bfloat16
F32 = mybir.dt.float32
AF = mybir.ActivationFunctionType
ALU = mybir.AluOpType

# ---- problem constants ----------------------------------------------------
DIM = 256
HEADS = 8
HD = 32
NTOK = 64
NW49 = 49
BATCH = 64
B_ = BATCH * NW49          # 3136
NCORES = 8
WPC = B_ // NCORES         # 392 windows per core
TPC = WPC * NTOK           # 25088 tokens per core
CHW = 8                    # windows per chunk
NCHUNK = WPC // CHW        # 49
SCALE = HD ** -0.5


def _np_bf16(a):
    return np.asarray(a, dtype=ml_dtypes.bfloat16)


# ---- host-side DPB MLP + relative-position tables --------------------------
def _host_rpb(inputs):
    """rpb[h, q, k] = p3[rel_idx[q, k], h] where p3 = DPB MLP(biases)."""
    f = lambda k: np.asarray(inputs[k], np.float32)
    biases = f("biases")            # (225, 2)
    eps = 1e-5

    def ln(x, g, b):
        m = x.mean(-1, keepdims=True)
        v = ((x - m) ** 2).mean(-1, keepdims=True)
        return (x - m) / np.sqrt(v + eps) * g + b

    p = biases @ f("pos_proj_w").T + f("pos_proj_b")
    p = np.maximum(ln(p, f("ln1_g"), f("ln1_b")), 0.0) @ f("fc1_w").T + f("fc1_b")
    p = np.maximum(ln(p, f("ln2_g"), f("ln2_b")), 0.0) @ f("fc2_w").T + f("fc2_b")
    p = np.maximum(ln(p, f("ln3_g"), f("ln3_b")), 0.0) @ f("fc3_w").T + f("fc3_b")
    # p: (225, HEADS)
    rel_idx = np.asarray(inputs["rel_idx"], np.int64)      # (64, 64)
    rpb = p[rel_idx]                                        # (q, k, h)
    return np.transpose(rpb, (2, 0, 1))                     # (h, q, k)


def _host_tables(inputs, mask_nonzero):
    """Fast path: tfast [128, 2048] bf16 with
         tfast[k2, r*512 + j*128 + hh*64 + q] = exp(rpb[hh*4+r, q, k2%64])
       (k duplicated over the two 64-partition window halves, replicated
       over j so the multiply AP is non-broadcast -> DVE 2x mode).
       Mask path: etmask [64, 49*4*128] bf16 with
         etmask[k, (t*4 + r)*128 + hh*64 + q] = exp(rpb + mask[t])."""
    rpb = _host_rpb(inputs)                                 # (h, q, k)
    if not mask_nonzero:
        t = np.empty((64, 2048), np.float32)
        for r in range(4):
            for hh in range(2):
                h = hh * 4 + r
                e = np.exp(rpb[h].T)                        # (k, q)
                for j in range(4):
                    t[:, r * 512 + j * 128 + hh * 64:
                         r * 512 + j * 128 + hh * 64 + 64] = e
        return _np_bf16(np.concatenate([t, t], axis=0)), None
    mask = np.asarray(inputs["mask"], np.float32)           # (49, 64, 64)
    em = np.empty((64, NW49 * 4 * 128), np.float32)
    for t49 in range(NW49):
        for r in range(4):
            for hh in range(2):
                h = hh * 4 + r
                e = np.exp(rpb[h] + mask[t49]).T            # (k, q)
                base = (t49 * 4 + r) * 128 + hh * 64
                em[:, base:base + 64] = e
    return None, _np_bf16(em)


# ---- device kernel builder -------------------------------------------------
def _build(mask_nonzero, qkvb_nonzero, projb_nonzero, nchunk=NCHUNK):
    nc = bacc.Bacc("TRN2", target_bir_lowering=False, debug=False)

    x_d = nc.dram_tensor("x", (2, 128, TPC), BF16, kind="ExternalInput")
    y_d = nc.dram_tensor("y", (TPC, DIM), BF16, kind="ExternalOutput")
    wqk_d = nc.dram_tensor("wqk", (2, 128, 512), BF16, kind="ExternalInput")
    wv_d = nc.dram_tensor("wv", (2, 128, 256), BF16, kind="ExternalInput")
    pw_d = nc.dram_tensor("pw", (2, 128, 256), BF16, kind="ExternalInput")
    vtmpl_d = nc.dram_tensor("vtmpl", (128, 2 * 4 * 264), BF16, kind="ExternalInput")
    if mask_nonzero:
        e_d = nc.dram_tensor("etab", (64, NW49 * 4 * 128), BF16, kind="ExternalInput")
    else:
        e_d = nc.dram_tensor("etab", (128, 2048), BF16, kind="ExternalInput")
    if qkvb_nonzero:
        qkb_d = nc.dram_tensor("qkb", (4, 128), F32, kind="ExternalInput")
        vb_d = nc.dram_tensor("vb", (128, 264), BF16, kind="ExternalInput")
    if projb_nonzero:
        yb_d = nc.dram_tensor("yb", (128, 256), F32, kind="ExternalInput")

    with tile.TileContext(nc) as tc:
        with (
            tc.tile_pool(name="setup", bufs=1) as setup,
            tc.tile_pool(name="xts", bufs=3) as xts,
            tc.tile_pool(name="qks", bufs=3) as qks,
            tc.tile_pool(name="ets", bufs=2) as ets,
            tc.tile_pool(name="avqs", bufs=2) as avqs,
            tc.tile_pool(name="avs", bufs=2) as avs,
            tc.tile_pool(name="ys", bufs=3) as ysp,
            tc.tile_pool(name="dst", bufs=4) as dst,
            tc.tile_pool(name="pp_a", bufs=2, space="PSUM") as pp_a,
            tc.tile_pool(name="pp_sc", bufs=2, space="PSUM") as pp_sc,
            tc.tile_pool(name="pp_av", bufs=1, space="PSUM") as pp_av,
        ):
            # ---- one-time setup ----
            ident = setup.tile([128, 128], BF16)
            make_identity(nc, ident)

            wqk = setup.tile([128, 2, 512], BF16)
            nc.gpsimd.dma_start(
                out=wqk,
                in_=bass.AP(tensor=wqk_d, offset=0,
                            ap=[[512, 128], [128 * 512, 2], [1, 512]]),
            )
            wv = setup.tile([128, 2, 256], BF16)
            nc.gpsimd.dma_start(
                out=wv,
                in_=bass.AP(tensor=wv_d, offset=0,
                            ap=[[256, 128], [128 * 256, 2], [1, 256]]),
            )
            pw = setup.tile([128, 2, 256], BF16)
            nc.gpsimd.dma_start(
                out=pw,
                in_=bass.AP(tensor=pw_d, offset=0,
                            ap=[[256, 128], [128 * 256, 2], [1, 256]]),
            )
            # v_aug: persistent double-buffered (h, 33)-block v staging with
            # pre-seeded ones columns (from the host template)
            v_aug = setup.tile([128, 2, 4, 264], BF16)
            nc.gpsimd.dma_start(out=v_aug, in_=vtmpl_d.ap())
            if mask_nonzero:
                etab = setup.tile([64, NW49 * 4 * 128], BF16)
                nc.gpsimd.dma_start(out=etab, in_=e_d.ap())
            else:
                etab = setup.tile([128, 2048], BF16)
                nc.gpsimd.dma_start(out=etab, in_=e_d.ap())
            if qkvb_nonzero:
                qkb = setup.tile([128, 4], F32)
                nc.gpsimd.dma_start(
                    out=qkb,
                    in_=bass.AP(tensor=qkb_d, offset=0, ap=[[1, 128], [128, 4]]),
                )
                vb = setup.tile([128, 264], BF16)
                nc.gpsimd.dma_start(out=vb, in_=vb_d.ap())
            if projb_nonzero:
                yb = setup.tile([128, 256], F32)
                nc.gpsimd.dma_start(out=yb, in_=yb_d.ap())

            # ---- software-pipelined main loop (8 windows / 512 tok per chunk)
            # PE order per iteration: qkv(c) | AV(c-1) | scores(c) | O^T+proj(c-1)
            # so the PE never idles on the chunk-tail DVE chain (keeps HAM warm).

            def head1(c):
                tok0 = c * 512
                par = c % 2
                st = {}
                xt = xts.tile([128, 2, 512], BF16)
                nc.sync.dma_start(
                    out=xt,
                    in_=bass.AP(tensor=x_d, offset=tok0,
                                ap=[[TPC, 128], [128 * TPC, 2], [1, 512]]),
                )
                # qkT = Wqk.T @ xT : 4 m-tiles (q0 q1 k0 k1)
                qk = qks.tile([128, 4, 512], BF16)
                for m in (0, 2, 1, 3):
                    qkp = pp_a.tile([128, 512], F32, name="qkp", tag="mm")
                    for kk in range(2):
                        nc.tensor.matmul(
                            qkp,
                            wqk[:, kk, 128 * m:128 * (m + 1)],
                            xt[:, kk, :],
                            start=(kk == 0),
                            stop=(kk == 1),
                        )
                    if qkvb_nonzero:
                        if m < 2:
                            nc.scalar.activation(
                                qk[:, m, :], qkp, AF.Identity,
                                bias=qkb[:, m:m + 1],
                            )
                        else:
                            nc.vector.tensor_scalar_add(
                                qk[:, m, :], qkp, qkb[:, m:m + 1]
                            )
                    else:
                        if m < 2:
                            nc.scalar.copy(qk[:, m, :], qkp)
                        else:
                            nc.vector.tensor_copy(qk[:, m, :], qkp)

                # v (token-major) -> v_aug (h, 33)-blocks, ones col persists
                for jp in range(2):
                    vp = pp_a.tile([128, 512], F32, name="vp", tag="mm")
                    for j2 in range(2):
                        j = jp * 2 + j2
                        for kk in range(2):
                            nc.tensor.matmul(
                                vp[:, 256 * j2:256 * (j2 + 1)],
                                xt[:, kk, 128 * j:128 * (j + 1)],
                                wv[:, kk, :],
                                start=(kk == 0),
                                stop=(kk == 1),
                            )
                    vdst = bass.AP(
                        tensor=v_aug.tensor,
                        offset=v_aug.offset + par * (4 * 264) + jp * 2 * 264,
                        ap=[v_aug.ap[0], [264, 2], [33, 8], [1, 32]],
                    )
                    vsrc = vp.rearrange("p (j2 h d) -> p j2 h d", j2=2, h=8)
                    if qkvb_nonzero:
                        vb_ap = bass.AP(
                            tensor=vb.tensor, offset=vb.offset,
                            ap=[vb.ap[0], [0, 2], [33, 8], [1, 32]],
                        )
                        nc.vector.tensor_tensor(vdst, vsrc, vb_ap, ALU.add)
                    else:
                        nc.vector.tensor_copy(vdst, vsrc)
                st["qk"] = qk
                return st

            def head2(c, st, rs):
                qk = st["qk"]
                # scores^T: per-r psum [128 = 2win keys, 512 = (j, hh, q)];
                # r-serial so exp(r) frees its bank during r+1's matmuls
                if "et" in st:
                    et = st["et"]
                else:
                    et = ets.tile([128, 2048], BF16)
                for r in rs:
                    sc = pp_sc.tile([128, 512], F32, name=f"sc{r}", tag="sc")
                    scv = sc.rearrange("p (j hh q) -> p j hh q", j=4, hh=2)
                    for j in range(4):
                        for hh in range(2):
                            for win in range(2):
                                base = 128 * j + 64 * win
                                nc.tensor.matmul(
                                    scv[64 * win:64 * (win + 1), j, hh, :],
                                    qk[32 * r:32 * r + 32, 2 + hh,
                                       base:base + 64],
                                    qk[32 * r:32 * r + 32, hh,
                                       base:base + 64],
                                    tile_position=(32 * r, 64 * win),
                                )
                    nc.scalar.activation(
                        et[:, 512 * r:512 * (r + 1)], sc, AF.Exp
                    )

                # fold exp(rpb [+ mask]) multiplicatively
                if not mask_nonzero:
                    for r in rs:
                        sl = et[:, 512 * r:512 * (r + 1)]
                        nc.vector.tensor_tensor(
                            sl, sl, etab[:, 512 * r:512 * (r + 1)], ALU.mult
                        )
                else:
                    for w in range(CHW):
                        t49 = (c * CHW + w) % NW49
                        j, win = w // 2, w % 2
                        for r in rs:
                            sl = et[64 * win:64 * (win + 1),
                                    512 * r + 128 * j:512 * r + 128 * j + 128]
                            e_ap = bass.AP(
                                tensor=etab.tensor,
                                offset=etab.offset + (t49 * 4 + r) * 128,
                                ap=[etab.ap[0], [1, 128]],
                            )
                            nc.vector.tensor_tensor(sl, sl, e_ap, ALU.mult)
                st["et"] = et

            def tail_a(c, st):
                et = st["et"]
                par = c % 2
                # AV (+z): stationary E^T slices, stream v_aug (33-wide);
                # win innermost alternates PE row groups
                av = pp_av.tile([128, 2048], F32, name="av", tag="av")
                for r in range(4):
                    for j in range(4):
                        for hh in range(2):
                            h8 = hh * 4 + r
                            ebase = 512 * r + 128 * j + 64 * hh
                            for win in range(2):
                                nc.tensor.matmul(
                                    av[64 * win:64 * (win + 1),
                                       j * 512 + 33 * h8:j * 512 + 33 * h8 + 33],
                                    et[64 * win:64 * (win + 1), ebase:ebase + 64],
                                    v_aug[64 * win:64 * (win + 1), par, j,
                                          33 * h8:33 * h8 + 33],
                                    tile_position=(64 * win, 64 * win),
                                )

                # reciprocal of z (strided cols 512j + 33h + 32)
                rz = dst.tile([128, 4, 8], F32)
                z_ap = bass.AP(
                    tensor=av.tensor, offset=av.offset + 32,
                    ap=[av.ap[0], [512, 4], [33, 8]],
                )
                nc.vector.reciprocal_approx_fast(rz, z_ap)

                # normalize + evacuate to compact q-major bf16
                avq = avqs.tile([128, 1024], BF16)
                avq_v = avq.rearrange("p (j h d) -> p j h d", j=4, h=8)
                o_ap = bass.AP(
                    tensor=av.tensor, offset=av.offset,
                    ap=[av.ap[0], [512, 4], [33, 8], [1, 32]],
                )
                rz_ap = bass.AP(
                    tensor=rz.tensor, offset=rz.offset,
                    ap=[rz.ap[0], [8, 4], [1, 8], [0, 32]],
                )
                nc.vector.tensor_tensor(avq_v, o_ap, rz_ap, ALU.mult)
                st["avq"] = avq

            def tail_b1(c, st):
                avq = st["avq"]
                # transpose O back to feature-major for proj
                av_sb = avs.tile([128, 2, 512], BF16)
                for tp in range(2):
                    tr = pp_a.tile([128, 512], BF16, name=f"tr{tp}", tag="mm")
                    for i in range(4):
                        j = tp * 2 + i // 2
                        kk = i % 2
                        nc.tensor.transpose(
                            tr[:, 128 * i:128 * (i + 1)],
                            avq[:, 256 * j + 128 * kk:256 * j + 128 * kk + 128],
                            ident,
                        )
                    # tr cols = (j2, kk, q) -> av_sb[., kk, 128j + q]
                    tdst = bass.AP(
                        tensor=av_sb.tensor,
                        offset=av_sb.offset + tp * 256,
                        ap=[av_sb.ap[0], [128, 2], [512, 2], [1, 128]],
                    )
                    tsrc = tr.rearrange("p (j2 kk q) -> p j2 kk q", j2=2, kk=2)
                    if tp == 0:
                        nc.vector.tensor_copy(tdst, tsrc)
                    else:
                        nc.scalar.copy(tdst, tsrc)
                st["av_sb"] = av_sb

            def tail_b2(c, st):
                av_sb = st["av_sb"]
                tok0 = c * 512
                # proj
                y_sb = ysp.tile([128, 4, 256], BF16)
                for yp2 in range(2):
                    yp = pp_a.tile([128, 512], F32, name=f"yp{yp2}", tag="mm")
                    for l in range(2):
                        jj = yp2 * 2 + l
                        for kk in range(2):
                            nc.tensor.matmul(
                                yp[:, 256 * l:256 * (l + 1)],
                                av_sb[:, kk, 128 * jj:128 * (jj + 1)],
                                pw[:, kk, :],
                                start=(kk == 0),
                                stop=(kk == 1),
                            )
                    if projb_nonzero:
                        yb_ap = bass.AP(
                            tensor=yb.tensor, offset=yb.offset,
                            ap=[yb.ap[0], [0, 2], [1, 256]],
                        )
                        ydst = y_sb[:, 2 * yp2:2 * yp2 + 2, :]
                        nc.vector.tensor_tensor(
                            ydst, yp.rearrange("p (l f) -> p l f", l=2),
                            yb_ap, ALU.add,
                        )
                    else:
                        nc.scalar.copy(y_sb[:, 2 * yp2:2 * yp2 + 2, :], yp)

                nc.sync.dma_start(
                    out=bass.AP(tensor=y_d, offset=tok0 * DIM,
                                ap=[[DIM, 128], [128 * DIM, 4], [1, DIM]]),
                    in_=y_sb,
                )

            states = {0: head1(0)}
            head2(0, states[0], (0, 1))
            head2(0, states[0], (2, 3))
            for c in range(1, nchunk):
                states[c] = head1(c)
                tail_a(c - 1, states[c - 1])
                head2(c, states[c], (0, 1))
                tail_b1(c - 1, states[c - 1])
                head2(c, states[c], (2, 3))
                tail_b2(c - 1, states[c - 1])
                del states[c - 1]
            tail_a(nchunk - 1, states[nchunk - 1])
            tail_b1(nchunk - 1, states[nchunk - 1])
            tail_b2(nchunk - 1, states[nchunk - 1])

    nc.compile()
    return nc


# ---- execution --------------------------------------------------------------
_CACHE = {}


def _get_runner(mask_nonzero, qkvb_nonzero, projb_nonzero, nchunk=NCHUNK):
    key = (mask_nonzero, qkvb_nonzero, projb_nonzero, nchunk)
    if key in _CACHE:
        return _CACHE[key]

    nc = _build(mask_nonzero, qkvb_nonzero, projb_nonzero, nchunk)

    import jax
    import jax.numpy as jnp
    from jax.sharding import Mesh, PartitionSpec
    from jax.experimental.shard_map import shard_map
    from concourse import bass2jax
    from concourse.bass2jax import _bass_exec_p, install_neuronx_cc_hook

    install_neuronx_cc_hook()

    partition_name = (
        nc.partition_id_tensor.name if nc.partition_id_tensor else None
    )
    in_names, out_names, out_avals, zero_outs = [], [], [], []
    for alloc in nc.m.functions[0].allocations:
        if not isinstance(alloc, mybir.MemoryLocationSet):
            continue
        name = alloc.memorylocations[0].name
        if alloc.kind == "ExternalInput":
            if name != partition_name:
                in_names.append(name)
        elif alloc.kind == "ExternalOutput":
            shape = tuple(alloc.tensor_shape)
            dtype = mybir.dt.np(alloc.dtype)
            out_names.append(name)
            out_avals.append(jax.core.ShapedArray(shape, dtype))
            zero_outs.append(np.zeros(shape, dtype))
    n_params = len(in_names)
    n_outs = len(out_avals)
    all_in_names = list(in_names) + list(out_names)
    if partition_name is not None:
        all_in_names.append(partition_name)

    def _body(*args):
        operands = list(args)
        if partition_name is not None:
            operands.append(bass2jax.partition_id_tensor())
        outs = _bass_exec_p.bind(
            *operands,
            out_avals=tuple(out_avals),
            in_names=tuple(all_in_names),
            out_names=tuple(out_names),
            lowering_input_output_aliases=(),
            sim_require_finite=True,
            sim_require_nnan=True,
            nc=nc,
        )
        return tuple(outs)

    devices = jax.devices()[:NCORES]
    mesh = Mesh(np.asarray(devices), ("core",))
    donate = tuple(range(n_params, n_params + n_outs))
    sharded = jax.jit(
        shard_map(
            _body, mesh=mesh,
            in_specs=(PartitionSpec("core"),) * (n_params + n_outs),
            out_specs=(PartitionSpec("core"),) * n_outs,
            check_rep=False,
        ),
        donate_argnums=donate,
        keep_unused=True,
    )

    from jax.sharding import NamedSharding

    shard = NamedSharding(mesh, PartitionSpec("core"))
    zero_shapes = [
        ((NCORES * z.shape[0], *z.shape[1:]), z.dtype) for z in zero_outs
    ]
    make_zeros = jax.jit(
        lambda: tuple(jnp.zeros(s, d) for s, d in zero_shapes),
        out_shardings=(shard,) * n_outs,
    )

    def _concat(in_maps):
        return [
            np.concatenate([np.asarray(in_maps[c][nm]) for c in range(NCORES)], axis=0)
            for nm in in_names
        ]

    def run(in_maps):
        out_arrs = sharded(*_concat(in_maps), *make_zeros())
        out = np.asarray(out_arrs[out_names.index("y")])
        return out.reshape(NCORES, TPC, DIM)

    def bench(in_maps, iters=8):
        import time as _time

        dev_in = [jax.device_put(a, shard) for a in _concat(in_maps)]
        jax.block_until_ready(dev_in)
        outs = sharded(*dev_in, *make_zeros())
        jax.block_until_ready(outs)
        ts = []
        for _ in range(iters):
            t0 = _time.perf_counter()
            outs = sharded(*dev_in, *make_zeros())
            jax.block_until_ready(outs)
            ts.append(_time.perf_counter() - t0)
        return min(ts), ts

    def bench_repeat(in_maps, reps, iters=6):
        """Async-chain `reps` dispatches (output ping-pongs into the donated
        slot) and block once; median over iters."""
        import time as _time

        dev_in = [jax.device_put(a, shard) for a in _concat(in_maps)]
        jax.block_until_ready(dev_in)
        outs = sharded(*dev_in, *make_zeros())
        jax.block_until_ready(outs)
        ts = []
        for _ in range(iters):
            outs = sharded(*dev_in, *make_zeros())
            jax.block_until_ready(outs)
            t0 = _time.perf_counter()
            for _ in range(reps):
                outs = sharded(*dev_in, *outs)
            jax.block_until_ready(outs)
            ts.append(_time.perf_counter() - t0)
        ts.sort()
        return ts[len(ts) // 2], ts

    run.bench = bench
    run.bench_repeat = bench_repeat
    _CACHE[key] = run
    return run


def _prep_in_maps(inputs):
    x = np.asarray(inputs["x"], np.float32).reshape(B_ * NTOK, DIM)
    qkv_w = np.asarray(inputs["qkv_w"], np.float32)
    qkv_b = np.asarray(inputs["qkv_b"], np.float32)
    proj_w = np.asarray(inputs["proj_w"], np.float32)
    proj_b = np.asarray(inputs["proj_b"], np.float32)
    mask_nonzero = bool(np.any(np.asarray(inputs["mask"]) != 0))
    qkvb_nonzero = bool(np.any(qkv_b != 0))
    projb_nonzero = bool(np.any(proj_b != 0))

    wqk_f = qkv_w[:512].copy()
    wqk_f[:256] *= SCALE                       # fold q scale into Wq
    wqk = _np_bf16(wqk_f.T.reshape(2, 128, 512))
    wv = _np_bf16(qkv_w[512:].T.reshape(2, 128, 256))
    pw = _np_bf16(proj_w.T.reshape(2, 128, 256))

    tfast, etmask = _host_tables(inputs, mask_nonzero)
    etab = etmask if mask_nonzero else tfast

    # v_aug template: zeros with ones in the 33rd column of each head block
    vtmpl = np.zeros((128, 2, 4, 8, 33), np.float32)
    vtmpl[:, :, :, :, 32] = 1.0
    vtmpl = _np_bf16(vtmpl.reshape(128, 2 * 4 * 264))

    shared = {"wqk": wqk, "wv": wv, "pw": pw, "etab": etab, "vtmpl": vtmpl}
    if qkvb_nonzero:
        qkb_f = qkv_b[:512].copy()
        qkb_f[:256] *= SCALE
        shared["qkb"] = qkb_f.reshape(4, 128).astype(np.float32)
        vb_aug = np.zeros((128, 8, 33), np.float32)
        vb_aug[:, :, :32] = qkv_b[512:].reshape(1, 8, 32)
        shared["vb"] = _np_bf16(vb_aug.reshape(128, 264))
    if projb_nonzero:
        shared["yb"] = np.broadcast_to(proj_b, (128, 256)).copy().astype(np.float32)

    in_maps = []
    for c in range(NCORES):
        m = dict(shared)
        xs = x[c * TPC:(c + 1) * TPC]
        m["x"] = np.ascontiguousarray(_np_bf16(xs.T)).reshape(2, 128, TPC)
        in_maps.append(m)
    flags = (mask_nonzero, qkvb_nonzero, projb_nonzero)
    return in_maps, flags


def kernel(**inputs) -> np.ndarray:
    in_maps, flags = _prep_in_maps(inputs)
    run = _get_runner(*flags)
    out = run(in_maps)                          # (8, TPC, DIM) bf16
    return np.asarray(out, dtype=np.float32).reshape(B_, NTOK, DIM)
